# revision 37
# baseline (speedup 1.0000x reference)
"""Trainium2 Bass kernel for the FD (facilitation-depression) synapse layer.

Reference computes, per (b, h) lane, a sequential recurrence over T timesteps
with K=4 unrolled Euler substeps:

    Ca_diff = Ca - Ca_mu
    sig     = sigmoid(Ca_diff / Ca_sigma)
    temp    = P_rel_max*sig * R * I_t
    EPSC    = EPSC - dt*(EPSC / tau_EPSC + beta*temp)
    R       = R + dt*((k_min + k_delta*sig)*(1-R) - temp)
    Ca      = Ca + dt*(alpha*I_t - Ca_diff/tau_Ca)

Structure exploited (beyond the scan formulation of the predecessor):
  * Ca is a per-lane exponential moving average of u_t = dt*alpha*I_t +
    dt/tau_Ca*mu with per-timestep decay C1 = c1^4 in [0.19, 0.45]; it
    forgets its past in ~8 steps, so Ca'_t is computed (to ~1e-3) as an
    8-tap convolution of u -- a chain of 10 diagonal-weight matmuls on
    the otherwise-idle PE accumulating in PSUM.  The initial condition is
    injected exactly as a virtual u_{-1} = Ca'_0 column.
  * The substep sigmoid arguments comb_k = g_k*cap0 + u_t are 2 more
    diag-matmuls each; the ACT sigmoid reads the PSUM bank directly.
  * The Horner combination sacc = sum_k e1^{3-k} sr_k is 4 diag-matmuls
    with strided rhs (k-slices of sr), also on PE.
  * R keeps the DVE tensor_tensor_scan at substep granularity (data-
    dependent coefficients admit no cheaper form); Pt/Qt are DVE 4x-mode
    tensor_scalar ops from the packed bf16 sig tile (Qt partially on ACT
    for balance).
  * Engine balance: ACT sigmoids + affine I-transforms; PE convolutions;
    DVE scan/TS/racc + most sr products; GPSIMD vsig + a share of sr.

Sharding: batch 32 -> 4 samples per core (pure data parallel). Per core the
4*512 = 2048 lanes are processed as 16 lane-batches of 128 partitions; time is
blocked at TB timesteps with scan-state carried across blocks.

Host side does all parameter transforms (including the diagonal PE weight
matrices) and the (b,t,h) <-> (lane, t) transposes in numpy.
"""

import numpy as np
from contextlib import ExitStack

import concourse.bass as bass
import concourse.mybir as mybir
import concourse.tile as tile
from concourse.bass_utils import run_bass_kernel_spmd

f32 = mybir.dt.float32
bf16 = mybir.dt.bfloat16
AF = mybir.ActivationFunctionType
OP = mybir.AluOpType

B, T, H = 32, 2048, 512
K = 4               # ode substeps per timestep
NCORES = 8
BPC = B // NCORES   # batch per core (4)
GH = H // 128       # h-groups of 128 (4)
NLB = BPC * GH      # lane batches per core (16)
PD = 128            # partitions
TB = 512            # timesteps per block
J = 8               # Ca convolution taps (C1^J <= 0.45^8 ~ 2e-3)
NPAR = 20
NW = 19             # diag weight slots per h-group:
                    #   0..9   conv taps diag(C1^j)
                    #   10..11 diag(SC1*G1), diag(SC2*G2); 12 diag(G3-1)
                    #   13..16 sacc diag(e1^{3-k}), slot 16 = identity
                    #   17..18 diag(SC1), diag(SC2)
SRGP = 0            # sr instances (of 20) steered to GPSIMD
QTACT = 0           # Qt instances (of 20) steered to ACT
RACCACT = 20        # racc instances (of 20) routed via ACT sacc-evacuation
RACCGP = 18         # of the ACT-evacuated raccs, how many go to GPSIMD
VSFILL = 2          # first units whose vsig runs on the (idle) DVE

(C1, G1, G2, G3, SC0, SC1, SC2, SC3, BIAS, UC,
 UA, SV, AV, CP, QM, QA, SW2, E1, E14, CA0) = range(NPAR)


def build_program(Tn=T, tb=TB, nlb=NLB, n_devices=NCORES):
    """Build the Bass program (SPMD; same program on every core)."""
    nblk = Tn // tb
    S = K * tb
    nc = bass.Bass("TRN2", target_bir_lowering=False, debug=False,
                   num_devices=n_devices)
    I_d = nc.dram_tensor("i_ca", [nlb, PD, Tn], f32, kind="ExternalInput").ap()
    par_d = nc.dram_tensor("par", [PD, nlb * NPAR], f32,
                           kind="ExternalInput").ap()
    dw_d = nc.dram_tensor("dw", [GH, PD, NW * PD], bf16,
                          kind="ExternalInput").ap()
    O_d = [[nc.dram_tensor(f"epsc_{lb}_{blk}", [PD, tb], f32,
                           kind="ExternalOutput").ap()
            for blk in range(nblk)] for lb in range(nlb)]

    with ExitStack() as ctx:
        tc = ctx.enter_context(tile.TileContext(nc))
        vpool = ctx.enter_context(tc.tile_pool(name="vhand", bufs=2))
        wpool = ctx.enter_context(tc.tile_pool(name="w2hand", bufs=3))
        spool = ctx.enter_context(tc.tile_pool(name="sig", bufs=7))
        cspool = ctx.enter_context(tc.tile_pool(name="cap0s", bufs=2))
        bpool = ctx.enter_context(tc.tile_pool(name="bshort", bufs=4))
        cpool = ctx.enter_context(tc.tile_pool(name="bcarry", bufs=4))
        rpool = ctx.enter_context(tc.tile_pool(name="rbig", bufs=2))
        ipool = ctx.enter_context(tc.tile_pool(name="inp", bufs=2))
        upool = ctx.enter_context(tc.tile_pool(name="ufull", bufs=2))
        ppool = ctx.enter_context(tc.tile_pool(name="par", bufs=1))
        ps1 = ctx.enter_context(tc.tile_pool(name="ps1", bufs=3, space="PSUM"))
        ps2 = ctx.enter_context(tc.tile_pool(name="ps2", bufs=2, space="PSUM"))
        ps3 = ctx.enter_context(tc.tile_pool(name="ps3", bufs=1, space="PSUM"))

        par = ppool.tile([PD, nlb * NPAR], f32, tag="par")
        nc.sync.dma_start(par[:], par_d)
        dw = ppool.tile([PD, GH * NW * PD], bf16, tag="dw")
        for g in range(GH):
            nc.sync.dma_start(dw[:, g * NW * PD:(g + 1) * NW * PD], dw_d[g])

        def wmat(g, i):
            return dw[:, (g * NW + i) * PD:(g * NW + i + 1) * PD]

        itile_lbs = {}
        ufull_lbs = {}
        vw_lbs = {}
        prev_rb = {}
        prev_e = {}

        def pcol_of(lb):
            return lambda i: par[:, lb * NPAR + i:lb * NPAR + i + 1]

        def fetch_itile(lb):
            itile_lb = ipool.tile([PD, Tn], f32, tag="itile")
            nc.sync.dma_start(itile_lb[:], I_d[lb])
            itile_lbs[lb] = itile_lb

        def stage_a0(lb, blk):
            """Half-lb u/V/W2 ACT transforms, spread over the lb's 4 steps
            (u first: the PE conv consumes it immediately; V by a4, W2 by
            bh).  Prefetches the next lb's itile at blk 0."""
            pcol = pcol_of(lb)
            t0 = blk * tb
            HT = Tn // 2
            if blk == 0:
                if lb == 0:
                    fetch_itile(0)
                if lb + 1 < nlb:
                    fetch_itile(lb + 1)
                uf = upool.tile([PD, J + Tn], bf16, tag="ufull")
                nc.vector.memset(uf[:, 0:J - 1], 0.0)
                nc.vector.tensor_copy(uf[:, J - 1:J], pcol(CA0))
                ufull_lbs[lb] = uf
                Vt = vpool.tile([PD, Tn], bf16, tag="V")
                W2t = wpool.tile([PD, Tn], bf16, tag="W2")
                vw_lbs[lb] = (Vt, W2t)
            itile = itile_lbs[lb]
            uf = ufull_lbs[lb]
            V, W2 = vw_lbs[lb]
            half = lambda h: slice(h * HT, (h + 1) * HT)
            if blk == 0:
                nc.scalar.activation(uf[:, J:J + HT], itile[:, half(0)],
                                     AF.Identity, bias=pcol(UA),
                                     scale=pcol(UC))
            elif blk == 1:
                nc.scalar.activation(uf[:, J + HT:J + Tn], itile[:, half(1)],
                                     AF.Identity, bias=pcol(UA),
                                     scale=pcol(UC))
                nc.scalar.activation(V[:, half(0)], itile[:, half(0)],
                                     AF.Identity, bias=pcol(AV),
                                     scale=pcol(SV))
            elif blk == 2:
                nc.scalar.activation(V[:, half(1)], itile[:, half(1)],
                                     AF.Identity, bias=pcol(AV),
                                     scale=pcol(SV))
                nc.scalar.activation(W2[:, half(0)], itile[:, half(0)],
                                     AF.Copy, scale=pcol(SW2))
            else:
                nc.scalar.activation(W2[:, half(1)], itile[:, half(1)],
                                     AF.Copy, scale=pcol(SW2))
            return V[:, t0:t0 + tb], W2[:, t0:t0 + tb]

        def stage_a1(lb, blk, V, W2):
            """cap0 PE conv."""
            g = lb % GH
            t0 = blk * tb
            uf = ufull_lbs[lb]
            # cap0_t = sum_{j=1..J} C1^{j-1} u_{t-j}  (virtual u_{-1}=Ca'_0)
            cap0 = ps1.tile([PD, tb], f32, tag="cap0")
            for j in range(1, J + 1):
                nc.tensor.matmul(cap0[:], wmat(g, j - 1),
                                 uf[:, J + t0 - j:J + t0 - j + tb],
                                 start=(j == 1), stop=(j == J))
            return cap0, V, W2

        def stage_a2(lb, blk, cap0, V, W2):
            """sig0 + cap0 evacuation (ACT), comb PE matmuls."""
            pcol = pcol_of(lb)
            g = lb % GH
            t0 = blk * tb
            sig = spool.tile([PD, S], bf16, tag="sig")
            sig3 = sig[:].rearrange("p (t k) -> p t k", k=K)
            nc.scalar.activation(sig3[:, :, 0], cap0[:], AF.Sigmoid,
                                 bias=pcol(BIAS), scale=pcol(SC0))
            cap0s = cspool.tile([PD, tb], bf16, tag="cap0s")
            nc.scalar.activation(cap0s[:], cap0[:], AF.Copy)

            comb = ps2.tile([PD, 2, tb], f32, tag="comb")
            uslice = ufull_lbs[lb][:, J + t0:J + t0 + tb]
            for k in (1, 2):
                nc.tensor.matmul(comb[:, k - 1], wmat(g, 9 + k), cap0s[:],
                                 start=True, stop=False)
                nc.tensor.matmul(comb[:, k - 1], wmat(g, 16 + k), uslice,
                                 start=False, stop=True)
            # comb3 accumulates in place on top of cap0 (slot 12 = G3-1):
            # cap0 + (g3-1)*cap0s + u = g3*cap0 + u (after sig0/evac reads)
            nc.tensor.matmul(cap0[:], wmat(g, 12), cap0s[:],
                             start=False, stop=False, skip_group_check=True)
            nc.tensor.matmul(cap0[:], wmat(g, 16), uslice,
                             start=False, stop=True, skip_group_check=True)
            return sig, sig3, comb, cap0, V, W2

        def stage_a3(lb, blk, sig, sig3, comb, comb3, V, W2):
            """sig1..3 (ACT, all from PSUM; SC1/SC2 folded into PE wts)."""
            pcol = pcol_of(lb)
            nc.scalar.activation(sig3[:, :, 1:3],
                                 comb[:].rearrange("p k t -> p t k"),
                                 AF.Sigmoid, bias=pcol(BIAS), scale=1.0)
            nc.scalar.activation(sig3[:, :, 3], comb3[:], AF.Sigmoid,
                                 bias=pcol(BIAS), scale=pcol(SC3))
            return sig, sig3, V, W2

        def stage_a4(lb, blk, sig, sig3, V, W2):
            """vsig (GPSIMD; DVE for the first units while the b-pipeline
            is still filling and the DVE is idle)."""
            u = blk + nblk * lb
            vsig = bpool.tile([PD, S], bf16, tag="vsig")
            veng = nc.vector if u < VSFILL else nc.gpsimd
            veng.tensor_mul(
                vsig[:].rearrange("p (t k) -> p t k", k=K),
                sig[:].rearrange("p (t k) -> p t k", k=K),
                V[:].unsqueeze(2).broadcast_to((PD, tb, K)))
            return sig, vsig, W2

        def stage_b1(lb, blk, sig, vsig, W2):
            """Pt, Qt, R scan, sr."""
            pcol = pcol_of(lb)
            u = blk + nblk * lb

            Pt = bpool.tile([PD, S], bf16, tag="Pt")
            nc.vector.tensor_scalar(Pt[:], vsig[:], -1.0, pcol(CP),
                                    OP.mult, OP.add)
            Qt = bpool.tile([PD, S], bf16, tag="Qt")
            if u % 20 < QTACT:
                nc.scalar.activation(Qt[:], sig[:], AF.Identity,
                                     bias=pcol(QA), scale=pcol(QM))
            else:
                nc.vector.tensor_scalar(Qt[:], sig[:], pcol(QM), pcol(QA),
                                        OP.mult, OP.add)

            if blk == 0:
                rbig = rpool.tile([PD, nblk * S + 1], bf16, tag="rbig")
                nc.vector.memset(rbig[:, 0:1], 1.0)
                prev_rb[lb] = rbig
            rbig = prev_rb[lb]
            s0 = blk * S
            nc.vector.tensor_tensor_scan(rbig[:, s0 + 1:s0 + S + 1], Pt[:],
                                         Qt[:], rbig[:, s0:s0 + 1],
                                         OP.mult, OP.add)
            return sig, rbig[:, s0:s0 + S], W2

        def stage_b1x(lb, blk, sig, rsh, W2):
            """sr product, one step after the R scan completes."""
            u = blk + nblk * lb
            sr = bpool.tile([PD, S], bf16, tag="sr")
            seng = nc.gpsimd if u % 20 < SRGP else nc.vector
            seng.tensor_mul(sr[:], sig[:], rsh)
            return sr, W2

        def stage_bh(lb, blk, sr, W2):
            """sacc = sum_k e1^{3-k} sr_k on PE; racc = W2*sacc on DVE."""
            g = lb % GH
            u = blk + nblk * lb
            sacc = ps3.tile([PD, tb], f32, tag="sacc")
            srk = sr[:].rearrange("p (t k) -> p t k", k=K)
            for k in range(K):
                nc.tensor.matmul(sacc[:], wmat(g, 13 + k), srk[:, :, k],
                                 start=(k == 0), stop=(k == K - 1))
            racc = cpool.tile([PD, tb], bf16, tag="racc")
            if u % 20 < RACCACT:
                saccs = cpool.tile([PD, tb], bf16, tag="saccs")
                nc.scalar.activation(saccs[:], sacc[:], AF.Copy)
                reng = nc.gpsimd if u % 20 < RACCGP else nc.vector
                reng.tensor_tensor(racc[:], W2[:], saccs[:], OP.mult)
            else:
                nc.vector.tensor_tensor(racc[:], W2[:], sacc[:], OP.mult)
            return racc

        def stage_b2(lb, blk, racc):
            """EPSC scan + out-DMA."""
            pcol = pcol_of(lb)
            etile = cpool.tile([PD, tb], f32, tag="etile")
            einit = 0.0 if blk == 0 else prev_e[lb][:, tb - 1:tb]
            nc.vector.tensor_tensor_scan(
                etile[:], pcol(E14).to_broadcast((PD, tb)), racc[:],
                einit, OP.mult, OP.add)
            prev_e[lb] = etile
            nc.sync.dma_start(O_d[lb][blk][:], etile[:])

        # software pipeline; emission order per step is chosen so that in
        # each engine queue the instructions that free PSUM banks / unblock
        # the PE come first (ACT has no execution-queue lookahead):
        #   a2(i-2): sig0+evac [ACT], comb [PE]
        #   a3(i-3): sig123 [ACT], vsig [GP]
        #   a0(i):   u/V/W2 [ACT]
        #   a1(i-1): cap0 conv [PE]
        #   b1(i-5): Pt/Qt/scan/sr [DVE/GP]
        #   bh(i-7): sacc [PE], racc [DVE]
        #   b2(i-9): EPSC scan + DMA [DVE]
        units = [(lb, blk) for lb in range(nlb) for blk in range(nblk)]
        n = len(units)
        a0o, a1o, a2o, a3o, a4o, b1o, b1xo, bho = ({}, {}, {}, {}, {}, {},
                                                   {}, {})
        for i in range(n + 9):
            if 2 <= i <= n + 1:
                a2o[i - 2] = stage_a2(*units[i - 2], *a1o.pop(i - 2))
            if 3 <= i <= n + 2:
                a3o[i - 3] = stage_a3(*units[i - 3], *a2o.pop(i - 3))
            if i < n:
                a0o[i] = stage_a0(*units[i])
            if 1 <= i <= n:
                a1o[i - 1] = stage_a1(*units[i - 1], *a0o.pop(i - 1))
            if 4 <= i <= n + 3:
                a4o[i - 4] = stage_a4(*units[i - 4], *a3o.pop(i - 4))
            if 5 <= i <= n + 4:
                b1o[i - 5] = stage_b1(*units[i - 5], *a4o.pop(i - 5))
            if 6 <= i <= n + 5:
                b1xo[i - 6] = stage_b1x(*units[i - 6], *b1o.pop(i - 6))
            if 7 <= i <= n + 6:
                bho[i - 7] = stage_bh(*units[i - 7], *b1xo.pop(i - 7))
            if i >= 9:
                stage_b2(*units[i - 9], bho.pop(i - 9))

    import bass_rust
    bass_rust.generate_event_semaphores(nc)
    return nc


def derive_params(log_Ca_mu, log_Ca_sigma, log_tau_Ca, log_alpha, log_tau_EPSC,
                  log_beta, presigmoid_P_rel_max, log_k_recov_min,
                  log_k_recov_delta, ode_steps):
    """Host-side param math (fp64): ([H, NPAR] fp32, [GH, PD, NW*PD] fp32)."""
    d = np.float64
    dt = 1.0 / int(ode_steps)
    mu = np.exp(log_Ca_mu.astype(d))
    sigma = np.exp(log_Ca_sigma.astype(d))
    tau_Ca = np.exp(log_tau_Ca.astype(d))
    alpha = np.exp(log_alpha.astype(d))
    tau_E = np.exp(log_tau_EPSC.astype(d))
    beta = np.exp(log_beta.astype(d))
    Prm = 1.0 / (1.0 + np.exp(-presigmoid_P_rel_max.astype(d)))
    k_min = np.exp(log_k_recov_min.astype(d))
    k_delta = np.exp(log_k_recov_delta.astype(d))

    c1 = 1.0 - dt / tau_Ca
    S1 = np.ones_like(c1)
    S2 = 1.0 + c1
    S3 = 1.0 + c1 + c1 ** 2
    S4 = S3 + c1 ** 3
    e1 = 1.0 - dt / tau_E

    n = log_Ca_mu.shape[0]
    par = np.zeros((n, NPAR), np.float64)
    par[:, C1] = c1 ** 4
    par[:, G1] = c1 * S4 / S1
    par[:, G2] = c1 ** 2 * S4 / S2
    par[:, G3] = c1 ** 3 * S4 / S3
    par[:, SC0] = S4 / sigma
    par[:, SC1] = S1 / sigma
    par[:, SC2] = S2 / sigma
    par[:, SC3] = S3 / sigma
    par[:, BIAS] = -mu / sigma
    par[:, UC] = dt * alpha
    par[:, UA] = dt / tau_Ca * mu
    par[:, SV] = dt * Prm
    par[:, AV] = dt * k_delta
    par[:, CP] = 1.0 - dt * k_min
    par[:, QM] = dt * k_delta
    par[:, QA] = dt * k_min
    par[:, SW2] = -dt * beta * Prm
    par[:, E1] = e1
    par[:, E14] = e1 ** 4
    par[:, CA0] = mu / S4

    C1v = par[:, C1].reshape(GH, PD)
    Gv = np.stack([par[:, G1], par[:, G2], par[:, G3]], -1).reshape(GH, PD, 3)
    e1v = e1.reshape(GH, PD)
    dwm = np.zeros((GH, PD, NW * PD), np.float64)
    idx = np.arange(PD)
    for g in range(GH):
        for j in range(J):
            dwm[g, idx, j * PD + idx] = C1v[g] ** j          # slots 0..9
        SCv = np.stack([par[:, SC1], par[:, SC2]], -1).reshape(GH, PD, 2)
        for k in range(2):
            dwm[g, idx, (10 + k) * PD + idx] = SCv[g, :, k] * Gv[g, :, k]
            dwm[g, idx, (17 + k) * PD + idx] = SCv[g, :, k]
        dwm[g, idx, 12 * PD + idx] = Gv[g, :, 2] - 1.0       # slot 12: G3-1
        for k in range(K):
            dwm[g, idx, (13 + k) * PD + idx] = e1v[g] ** (3 - k)  # 13..16
    return par.astype(np.float32), dwm.astype(np.float32)


_PROG = None
LAST_RESULTS = None  # BassKernelResults of the most recent kernel() call


def _get_program():
    global _PROG
    if _PROG is None:
        _PROG = build_program()
    return _PROG


def _to_bf16(x):
    """fp32 -> bfloat16 numpy array (ml_dtypes)."""
    import ml_dtypes
    return x.astype(ml_dtypes.bfloat16)


def kernel(I_Ca, log_Ca_mu, log_Ca_sigma, log_tau_Ca, log_alpha, log_tau_EPSC,
           log_beta, presigmoid_P_rel_max, log_k_recov_min, log_k_recov_delta,
           ode_steps):
    assert int(ode_steps) == K, f"kernel hardcodes {K} substeps"
    I_Ca = np.asarray(I_Ca, np.float32)
    assert I_Ca.shape == (B, T, H)

    par_h, dwm = derive_params(
        np.asarray(log_Ca_mu), np.asarray(log_Ca_sigma), np.asarray(log_tau_Ca),
        np.asarray(log_alpha), np.asarray(log_tau_EPSC), np.asarray(log_beta),
        np.asarray(presigmoid_P_rel_max), np.asarray(log_k_recov_min),
        np.asarray(log_k_recov_delta), ode_steps)

    # lane-batch lb = b_local*GH + g holds lanes h = g*128 + p
    par_lb = par_h.reshape(GH, PD, NPAR)
    par_core = np.ascontiguousarray(
        np.broadcast_to(par_lb[None], (BPC, GH, PD, NPAR)).reshape(
            NLB, PD, NPAR).transpose(1, 0, 2).reshape(PD, NLB * NPAR))
    dw_core = _to_bf16(np.ascontiguousarray(dwm))

    nc = _get_program()
    in_maps = []
    for c in range(NCORES):
        Ic = I_Ca[c * BPC:(c + 1) * BPC]                    # [BPC, T, H]
        Ic = Ic.reshape(BPC, T, GH, PD).transpose(0, 2, 3, 1)
        in_maps.append({
            "i_ca": np.ascontiguousarray(Ic.reshape(NLB, PD, T)),
            "par": par_core,
            "dw": dw_core,
        })

    res = run_bass_kernel_spmd(nc, in_maps, core_ids=list(range(NCORES)))
    global LAST_RESULTS
    LAST_RESULTS = res
    nblk = T // TB
    out = np.empty((B, T, H), np.float32)
    for c in range(NCORES):
        Oc = np.stack([
            np.concatenate([res.results[c][f"epsc_{lb}_{blk}"]
                            for blk in range(nblk)], axis=1)
            for lb in range(NLB)])                          # [NLB, PD, T]
        Oc = Oc.reshape(BPC, GH, PD, T)
        out[c * BPC:(c + 1) * BPC] = Oc.transpose(0, 3, 1, 2).reshape(BPC, T, H)
    return out


# revision 42
# speedup vs baseline: 1.0041x; 1.0041x over previous
"""Trainium2 Bass kernel for the FD (facilitation-depression) synapse layer.

Reference computes, per (b, h) lane, a sequential recurrence over T timesteps
with K=4 unrolled Euler substeps:

    Ca_diff = Ca - Ca_mu
    sig     = sigmoid(Ca_diff / Ca_sigma)
    temp    = P_rel_max*sig * R * I_t
    EPSC    = EPSC - dt*(EPSC / tau_EPSC + beta*temp)
    R       = R + dt*((k_min + k_delta*sig)*(1-R) - temp)
    Ca      = Ca + dt*(alpha*I_t - Ca_diff/tau_Ca)

Structure exploited (beyond the scan formulation of the predecessor):
  * Ca is a per-lane exponential moving average of u_t = dt*alpha*I_t +
    dt/tau_Ca*mu with per-timestep decay C1 = c1^4 in [0.19, 0.45]; it
    forgets its past in ~8 steps, so Ca'_t is computed (to ~1e-3) as an
    8-tap convolution of u -- a chain of 10 diagonal-weight matmuls on
    the otherwise-idle PE accumulating in PSUM.  The initial condition is
    injected exactly as a virtual u_{-1} = Ca'_0 column.
  * The substep sigmoid arguments comb_k = g_k*cap0 + u_t are 2 more
    diag-matmuls each; the ACT sigmoid reads the PSUM bank directly.
  * The Horner combination sacc = sum_k e1^{3-k} sr_k is 4 diag-matmuls
    with strided rhs (k-slices of sr), also on PE.
  * R keeps the DVE tensor_tensor_scan at substep granularity (data-
    dependent coefficients admit no cheaper form); Pt/Qt are DVE 4x-mode
    tensor_scalar ops from the packed bf16 sig tile (Qt partially on ACT
    for balance).
  * Engine balance: ACT sigmoids + affine I-transforms; PE convolutions;
    DVE scan/TS/racc + most sr products; GPSIMD vsig + a share of sr.

Sharding: batch 32 -> 4 samples per core (pure data parallel). Per core the
4*512 = 2048 lanes are processed as 16 lane-batches of 128 partitions; time is
blocked at TB timesteps with scan-state carried across blocks.

Host side does all parameter transforms (including the diagonal PE weight
matrices) and the (b,t,h) <-> (lane, t) transposes in numpy.
"""

import numpy as np
from contextlib import ExitStack

import concourse.bass as bass
import concourse.mybir as mybir
import concourse.tile as tile
from concourse.bass_utils import run_bass_kernel_spmd

f32 = mybir.dt.float32
bf16 = mybir.dt.bfloat16
AF = mybir.ActivationFunctionType
OP = mybir.AluOpType

B, T, H = 32, 2048, 512
K = 4               # ode substeps per timestep
NCORES = 8
BPC = B // NCORES   # batch per core (4)
GH = H // 128       # h-groups of 128 (4)
NLB = BPC * GH      # lane batches per core (16)
PD = 128            # partitions
TB = 512            # timesteps per block
J = 8               # Ca convolution taps (C1^J <= 0.45^8 ~ 2e-3)
NPAR = 20
NW = 19             # diag weight slots per h-group:
                    #   0..9   conv taps diag(C1^j)
                    #   10..11 diag(SC1*G1), diag(SC2*G2); 12 diag(G3-1)
                    #   13..16 sacc diag(e1^{3-k}), slot 16 = identity
                    #   17..18 diag(SC1), diag(SC2)
SRGP = 0            # sr instances (of 20) steered to GPSIMD
QTACT = 0           # Qt instances (of 20) steered to ACT
RACCACT = 20        # racc instances (of 20) routed via ACT sacc-evacuation
RACCGP = 18         # of the ACT-evacuated raccs, how many go to GPSIMD
VSFILL = 2          # first units whose vsig runs on the (idle) DVE
UVWDVE = 1          # first lbs whose u/V/W2 run on the (idle) DVE
EVACDVE = 2         # first units whose cap0s evacuation runs on the DVE

(C1, G1, G2, G3, SC0, SC1, SC2, SC3, BIAS, UC,
 UA, SV, AV, CP, QM, QA, SW2, E1, E14, CA0) = range(NPAR)


def build_program(Tn=T, tb=TB, nlb=NLB, n_devices=NCORES):
    """Build the Bass program (SPMD; same program on every core)."""
    nblk = Tn // tb
    S = K * tb
    nc = bass.Bass("TRN2", target_bir_lowering=False, debug=False,
                   num_devices=n_devices)
    I_d = nc.dram_tensor("i_ca", [nlb, PD, Tn], f32, kind="ExternalInput").ap()
    par_d = nc.dram_tensor("par", [PD, nlb * NPAR], f32,
                           kind="ExternalInput").ap()
    dw_d = nc.dram_tensor("dw", [GH, PD, NW * PD], bf16,
                          kind="ExternalInput").ap()
    O_d = [[nc.dram_tensor(f"epsc_{lb}_{blk}", [PD, tb], f32,
                           kind="ExternalOutput").ap()
            for blk in range(nblk)] for lb in range(nlb)]

    with ExitStack() as ctx:
        tc = ctx.enter_context(tile.TileContext(nc))
        vpool = ctx.enter_context(tc.tile_pool(name="vhand", bufs=2))
        wpool = ctx.enter_context(tc.tile_pool(name="w2hand", bufs=3))
        spool = ctx.enter_context(tc.tile_pool(name="sig", bufs=7))
        cspool = ctx.enter_context(tc.tile_pool(name="cap0s", bufs=2))
        bpool = ctx.enter_context(tc.tile_pool(name="bshort", bufs=4))
        cpool = ctx.enter_context(tc.tile_pool(name="bcarry", bufs=4))
        rpool = ctx.enter_context(tc.tile_pool(name="rbig", bufs=2))
        ipool = ctx.enter_context(tc.tile_pool(name="inp", bufs=2))
        upool = ctx.enter_context(tc.tile_pool(name="ufull", bufs=2))
        ppool = ctx.enter_context(tc.tile_pool(name="par", bufs=1))
        ps1 = ctx.enter_context(tc.tile_pool(name="ps1", bufs=3, space="PSUM"))
        ps2 = ctx.enter_context(tc.tile_pool(name="ps2", bufs=2, space="PSUM"))
        ps3 = ctx.enter_context(tc.tile_pool(name="ps3", bufs=1, space="PSUM"))

        par = ppool.tile([PD, nlb * NPAR], f32, tag="par")
        nc.sync.dma_start(par[:], par_d)
        dw = ppool.tile([PD, GH * NW * PD], bf16, tag="dw")
        for g in range(GH):
            nc.sync.dma_start(dw[:, g * NW * PD:(g + 1) * NW * PD], dw_d[g])

        def wmat(g, i):
            return dw[:, (g * NW + i) * PD:(g * NW + i + 1) * PD]

        itile_lbs = {}
        ufull_lbs = {}
        vw_lbs = {}
        prev_rb = {}
        prev_e = {}

        def pcol_of(lb):
            return lambda i: par[:, lb * NPAR + i:lb * NPAR + i + 1]

        def fetch_itile(lb):
            itile_lb = ipool.tile([PD, Tn], f32, tag="itile")
            nc.sync.dma_start(itile_lb[:], I_d[lb])
            itile_lbs[lb] = itile_lb

        def stage_a0(lb, blk):
            """Half-lb u/V/W2 ACT transforms, spread over the lb's 4 steps
            (u first: the PE conv consumes it immediately; V by a4, W2 by
            bh).  Prefetches the next lb's itile at blk 0."""
            pcol = pcol_of(lb)
            t0 = blk * tb
            HT = Tn // 2
            if blk == 0:
                if lb == 0:
                    fetch_itile(0)
                if lb + 1 < nlb:
                    fetch_itile(lb + 1)
                uf = upool.tile([PD, J + Tn], bf16, tag="ufull")
                nc.vector.memset(uf[:, 0:J - 1], 0.0)
                nc.vector.tensor_copy(uf[:, J - 1:J], pcol(CA0))
                ufull_lbs[lb] = uf
                Vt = vpool.tile([PD, Tn], bf16, tag="V")
                W2t = wpool.tile([PD, Tn], bf16, tag="W2")
                vw_lbs[lb] = (Vt, W2t)
            itile = itile_lbs[lb]
            uf = ufull_lbs[lb]
            V, W2 = vw_lbs[lb]
            half = lambda h: slice(h * HT, (h + 1) * HT)

            def aff(out, in_, ci, ca):
                if lb < UVWDVE:
                    nc.vector.tensor_scalar(out, in_, pcol(ci), pcol(ca),
                                            OP.mult, OP.add)
                else:
                    nc.scalar.activation(out, in_, AF.Identity,
                                         bias=pcol(ca), scale=pcol(ci))

            def w2p(out, in_):
                if lb < UVWDVE:
                    nc.vector.tensor_scalar(out, in_, pcol(SW2), 0.0,
                                            OP.mult, OP.add)
                else:
                    nc.scalar.activation(out, in_, AF.Copy, scale=pcol(SW2))

            if blk == 0:
                aff(uf[:, J:J + HT], itile[:, half(0)], UC, UA)
            elif blk == 1:
                aff(uf[:, J + HT:J + Tn], itile[:, half(1)], UC, UA)
                aff(V[:, half(0)], itile[:, half(0)], SV, AV)
            elif blk == 2:
                aff(V[:, half(1)], itile[:, half(1)], SV, AV)
                w2p(W2[:, half(0)], itile[:, half(0)])
            else:
                w2p(W2[:, half(1)], itile[:, half(1)])
            return V[:, t0:t0 + tb], W2[:, t0:t0 + tb]

        def stage_a1(lb, blk, V, W2):
            """cap0 PE conv."""
            g = lb % GH
            t0 = blk * tb
            uf = ufull_lbs[lb]
            # cap0_t = sum_{j=1..J} C1^{j-1} u_{t-j}  (virtual u_{-1}=Ca'_0)
            cap0 = ps1.tile([PD, tb], f32, tag="cap0")
            for j in range(1, J + 1):
                nc.tensor.matmul(cap0[:], wmat(g, j - 1),
                                 uf[:, J + t0 - j:J + t0 - j + tb],
                                 start=(j == 1), stop=(j == J))
            return cap0, V, W2

        def stage_a2(lb, blk, cap0, V, W2):
            """sig0 + cap0 evacuation (ACT), comb PE matmuls."""
            pcol = pcol_of(lb)
            g = lb % GH
            t0 = blk * tb
            sig = spool.tile([PD, S], bf16, tag="sig")
            sig3 = sig[:].rearrange("p (t k) -> p t k", k=K)
            nc.scalar.activation(sig3[:, :, 0], cap0[:], AF.Sigmoid,
                                 bias=pcol(BIAS), scale=pcol(SC0))
            u_i = blk + nblk * lb
            cap0s = cspool.tile([PD, tb], bf16, tag="cap0s")
            if u_i < EVACDVE:
                nc.vector.tensor_copy(cap0s[:], cap0[:])
            else:
                nc.scalar.activation(cap0s[:], cap0[:], AF.Copy)

            comb = ps2.tile([PD, 2, tb], f32, tag="comb")
            uslice = ufull_lbs[lb][:, J + t0:J + t0 + tb]
            for k in (1, 2):
                nc.tensor.matmul(comb[:, k - 1], wmat(g, 9 + k), cap0s[:],
                                 start=True, stop=False)
                nc.tensor.matmul(comb[:, k - 1], wmat(g, 16 + k), uslice,
                                 start=False, stop=True)
            # comb3 accumulates in place on top of cap0 (slot 12 = G3-1):
            # cap0 + (g3-1)*cap0s + u = g3*cap0 + u (after sig0/evac reads)
            nc.tensor.matmul(cap0[:], wmat(g, 12), cap0s[:],
                             start=False, stop=False, skip_group_check=True)
            nc.tensor.matmul(cap0[:], wmat(g, 16), uslice,
                             start=False, stop=True, skip_group_check=True)
            return sig, sig3, comb, cap0, V, W2

        def stage_a3(lb, blk, sig, sig3, comb, comb3, V, W2):
            """sig1..3 (ACT, all from PSUM; SC1/SC2 folded into PE wts)."""
            pcol = pcol_of(lb)
            nc.scalar.activation(sig3[:, :, 1:3],
                                 comb[:].rearrange("p k t -> p t k"),
                                 AF.Sigmoid, bias=pcol(BIAS), scale=1.0)
            nc.scalar.activation(sig3[:, :, 3], comb3[:], AF.Sigmoid,
                                 bias=pcol(BIAS), scale=pcol(SC3))
            return sig, sig3, V, W2

        def stage_a4(lb, blk, sig, sig3, V, W2):
            """vsig (GPSIMD; DVE for the first units while the b-pipeline
            is still filling and the DVE is idle)."""
            u = blk + nblk * lb
            vsig = bpool.tile([PD, S], bf16, tag="vsig")
            veng = nc.vector if u < VSFILL else nc.gpsimd
            veng.tensor_mul(
                vsig[:].rearrange("p (t k) -> p t k", k=K),
                sig[:].rearrange("p (t k) -> p t k", k=K),
                V[:].unsqueeze(2).broadcast_to((PD, tb, K)))
            return sig, vsig, W2

        def stage_b1(lb, blk, sig, vsig, W2):
            """Pt, Qt, R scan, sr."""
            pcol = pcol_of(lb)
            u = blk + nblk * lb

            Pt = bpool.tile([PD, S], bf16, tag="Pt")
            nc.vector.tensor_scalar(Pt[:], vsig[:], -1.0, pcol(CP),
                                    OP.mult, OP.add)
            Qt = bpool.tile([PD, S], bf16, tag="Qt")
            if u % 20 < QTACT:
                nc.scalar.activation(Qt[:], sig[:], AF.Identity,
                                     bias=pcol(QA), scale=pcol(QM))
            else:
                nc.vector.tensor_scalar(Qt[:], sig[:], pcol(QM), pcol(QA),
                                        OP.mult, OP.add)

            if blk == 0:
                rbig = rpool.tile([PD, nblk * S + 1], bf16, tag="rbig")
                nc.vector.memset(rbig[:, 0:1], 1.0)
                prev_rb[lb] = rbig
            rbig = prev_rb[lb]
            s0 = blk * S
            nc.vector.tensor_tensor_scan(rbig[:, s0 + 1:s0 + S + 1], Pt[:],
                                         Qt[:], rbig[:, s0:s0 + 1],
                                         OP.mult, OP.add)
            return sig, rbig[:, s0:s0 + S], W2

        def stage_b1x(lb, blk, sig, rsh, W2):
            """sr product, one step after the R scan completes."""
            u = blk + nblk * lb
            sr = bpool.tile([PD, S], bf16, tag="sr")
            seng = nc.gpsimd if u % 20 < SRGP else nc.vector
            seng.tensor_mul(sr[:], sig[:], rsh)
            return sr, W2

        def stage_bh(lb, blk, sr, W2):
            """sacc = sum_k e1^{3-k} sr_k on PE; racc = W2*sacc on DVE."""
            g = lb % GH
            u = blk + nblk * lb
            sacc = ps3.tile([PD, tb], f32, tag="sacc")
            srk = sr[:].rearrange("p (t k) -> p t k", k=K)
            for k in range(K):
                nc.tensor.matmul(sacc[:], wmat(g, 13 + k), srk[:, :, k],
                                 start=(k == 0), stop=(k == K - 1))
            racc = cpool.tile([PD, tb], bf16, tag="racc")
            if u % 20 < RACCACT:
                saccs = cpool.tile([PD, tb], bf16, tag="saccs")
                nc.scalar.activation(saccs[:], sacc[:], AF.Copy)
                reng = nc.gpsimd if u % 20 < RACCGP else nc.vector
                reng.tensor_tensor(racc[:], W2[:], saccs[:], OP.mult)
            else:
                nc.vector.tensor_tensor(racc[:], W2[:], sacc[:], OP.mult)
            return racc

        def stage_b2(lb, blk, racc):
            """EPSC scan + out-DMA."""
            pcol = pcol_of(lb)
            etile = cpool.tile([PD, tb], f32, tag="etile")
            einit = 0.0 if blk == 0 else prev_e[lb][:, tb - 1:tb]
            nc.vector.tensor_tensor_scan(
                etile[:], pcol(E14).to_broadcast((PD, tb)), racc[:],
                einit, OP.mult, OP.add)
            prev_e[lb] = etile
            nc.sync.dma_start(O_d[lb][blk][:], etile[:])

        # software pipeline; emission order per step is chosen so that in
        # each engine queue the instructions that free PSUM banks / unblock
        # the PE come first (ACT has no execution-queue lookahead):
        #   a2(i-2): sig0+evac [ACT], comb [PE]
        #   a3(i-3): sig123 [ACT], vsig [GP]
        #   a0(i):   u/V/W2 [ACT]
        #   a1(i-1): cap0 conv [PE]
        #   b1(i-5): Pt/Qt/scan/sr [DVE/GP]
        #   bh(i-7): sacc [PE], racc [DVE]
        #   b2(i-9): EPSC scan + DMA [DVE]
        units = [(lb, blk) for lb in range(nlb) for blk in range(nblk)]
        n = len(units)
        a0o, a1o, a2o, a3o, a4o, b1o, b1xo, bho = ({}, {}, {}, {}, {}, {},
                                                   {}, {})
        for i in range(n + 9):
            if 2 <= i <= n + 1:
                a2o[i - 2] = stage_a2(*units[i - 2], *a1o.pop(i - 2))
            if 3 <= i <= n + 2:
                a3o[i - 3] = stage_a3(*units[i - 3], *a2o.pop(i - 3))
            if i < n:
                a0o[i] = stage_a0(*units[i])
            if 1 <= i <= n:
                a1o[i - 1] = stage_a1(*units[i - 1], *a0o.pop(i - 1))
            if 4 <= i <= n + 3:
                a4o[i - 4] = stage_a4(*units[i - 4], *a3o.pop(i - 4))
            if 5 <= i <= n + 4:
                b1o[i - 5] = stage_b1(*units[i - 5], *a4o.pop(i - 5))
            if 6 <= i <= n + 5:
                b1xo[i - 6] = stage_b1x(*units[i - 6], *b1o.pop(i - 6))
            if 7 <= i <= n + 6:
                bho[i - 7] = stage_bh(*units[i - 7], *b1xo.pop(i - 7))
            if i >= 9:
                stage_b2(*units[i - 9], bho.pop(i - 9))

    import bass_rust
    bass_rust.generate_event_semaphores(nc)
    return nc


def derive_params(log_Ca_mu, log_Ca_sigma, log_tau_Ca, log_alpha, log_tau_EPSC,
                  log_beta, presigmoid_P_rel_max, log_k_recov_min,
                  log_k_recov_delta, ode_steps):
    """Host-side param math (fp64): ([H, NPAR] fp32, [GH, PD, NW*PD] fp32)."""
    d = np.float64
    dt = 1.0 / int(ode_steps)
    mu = np.exp(log_Ca_mu.astype(d))
    sigma = np.exp(log_Ca_sigma.astype(d))
    tau_Ca = np.exp(log_tau_Ca.astype(d))
    alpha = np.exp(log_alpha.astype(d))
    tau_E = np.exp(log_tau_EPSC.astype(d))
    beta = np.exp(log_beta.astype(d))
    Prm = 1.0 / (1.0 + np.exp(-presigmoid_P_rel_max.astype(d)))
    k_min = np.exp(log_k_recov_min.astype(d))
    k_delta = np.exp(log_k_recov_delta.astype(d))

    c1 = 1.0 - dt / tau_Ca
    S1 = np.ones_like(c1)
    S2 = 1.0 + c1
    S3 = 1.0 + c1 + c1 ** 2
    S4 = S3 + c1 ** 3
    e1 = 1.0 - dt / tau_E

    n = log_Ca_mu.shape[0]
    par = np.zeros((n, NPAR), np.float64)
    par[:, C1] = c1 ** 4
    par[:, G1] = c1 * S4 / S1
    par[:, G2] = c1 ** 2 * S4 / S2
    par[:, G3] = c1 ** 3 * S4 / S3
    par[:, SC0] = S4 / sigma
    par[:, SC1] = S1 / sigma
    par[:, SC2] = S2 / sigma
    par[:, SC3] = S3 / sigma
    par[:, BIAS] = -mu / sigma
    par[:, UC] = dt * alpha
    par[:, UA] = dt / tau_Ca * mu
    par[:, SV] = dt * Prm
    par[:, AV] = dt * k_delta
    par[:, CP] = 1.0 - dt * k_min
    par[:, QM] = dt * k_delta
    par[:, QA] = dt * k_min
    par[:, SW2] = -dt * beta * Prm
    par[:, E1] = e1
    par[:, E14] = e1 ** 4
    par[:, CA0] = mu / S4

    C1v = par[:, C1].reshape(GH, PD)
    Gv = np.stack([par[:, G1], par[:, G2], par[:, G3]], -1).reshape(GH, PD, 3)
    e1v = e1.reshape(GH, PD)
    dwm = np.zeros((GH, PD, NW * PD), np.float64)
    idx = np.arange(PD)
    for g in range(GH):
        for j in range(J):
            dwm[g, idx, j * PD + idx] = C1v[g] ** j          # slots 0..9
        SCv = np.stack([par[:, SC1], par[:, SC2]], -1).reshape(GH, PD, 2)
        for k in range(2):
            dwm[g, idx, (10 + k) * PD + idx] = SCv[g, :, k] * Gv[g, :, k]
            dwm[g, idx, (17 + k) * PD + idx] = SCv[g, :, k]
        dwm[g, idx, 12 * PD + idx] = Gv[g, :, 2] - 1.0       # slot 12: G3-1
        for k in range(K):
            dwm[g, idx, (13 + k) * PD + idx] = e1v[g] ** (3 - k)  # 13..16
    return par.astype(np.float32), dwm.astype(np.float32)


_PROG = None
LAST_RESULTS = None  # BassKernelResults of the most recent kernel() call


def _get_program():
    global _PROG
    if _PROG is None:
        _PROG = build_program()
    return _PROG


def _to_bf16(x):
    """fp32 -> bfloat16 numpy array (ml_dtypes)."""
    import ml_dtypes
    return x.astype(ml_dtypes.bfloat16)


def kernel(I_Ca, log_Ca_mu, log_Ca_sigma, log_tau_Ca, log_alpha, log_tau_EPSC,
           log_beta, presigmoid_P_rel_max, log_k_recov_min, log_k_recov_delta,
           ode_steps):
    assert int(ode_steps) == K, f"kernel hardcodes {K} substeps"
    I_Ca = np.asarray(I_Ca, np.float32)
    assert I_Ca.shape == (B, T, H)

    par_h, dwm = derive_params(
        np.asarray(log_Ca_mu), np.asarray(log_Ca_sigma), np.asarray(log_tau_Ca),
        np.asarray(log_alpha), np.asarray(log_tau_EPSC), np.asarray(log_beta),
        np.asarray(presigmoid_P_rel_max), np.asarray(log_k_recov_min),
        np.asarray(log_k_recov_delta), ode_steps)

    # lane-batch lb = b_local*GH + g holds lanes h = g*128 + p
    par_lb = par_h.reshape(GH, PD, NPAR)
    par_core = np.ascontiguousarray(
        np.broadcast_to(par_lb[None], (BPC, GH, PD, NPAR)).reshape(
            NLB, PD, NPAR).transpose(1, 0, 2).reshape(PD, NLB * NPAR))
    dw_core = _to_bf16(np.ascontiguousarray(dwm))

    nc = _get_program()
    in_maps = []
    for c in range(NCORES):
        Ic = I_Ca[c * BPC:(c + 1) * BPC]                    # [BPC, T, H]
        Ic = Ic.reshape(BPC, T, GH, PD).transpose(0, 2, 3, 1)
        in_maps.append({
            "i_ca": np.ascontiguousarray(Ic.reshape(NLB, PD, T)),
            "par": par_core,
            "dw": dw_core,
        })

    res = run_bass_kernel_spmd(nc, in_maps, core_ids=list(range(NCORES)))
    global LAST_RESULTS
    LAST_RESULTS = res
    nblk = T // TB
    out = np.empty((B, T, H), np.float32)
    for c in range(NCORES):
        Oc = np.stack([
            np.concatenate([res.results[c][f"epsc_{lb}_{blk}"]
                            for blk in range(nblk)], axis=1)
            for lb in range(NLB)])                          # [NLB, PD, T]
        Oc = Oc.reshape(BPC, GH, PD, T)
        out[c * BPC:(c + 1) * BPC] = Oc.transpose(0, 3, 1, 2).reshape(BPC, T, H)
    return out


# revision 48
# speedup vs baseline: 1.0177x; 1.0135x over previous
"""Trainium2 Bass kernel for the FD (facilitation-depression) synapse layer.

Reference computes, per (b, h) lane, a sequential recurrence over T timesteps
with K=4 unrolled Euler substeps:

    Ca_diff = Ca - Ca_mu
    sig     = sigmoid(Ca_diff / Ca_sigma)
    temp    = P_rel_max*sig * R * I_t
    EPSC    = EPSC - dt*(EPSC / tau_EPSC + beta*temp)
    R       = R + dt*((k_min + k_delta*sig)*(1-R) - temp)
    Ca      = Ca + dt*(alpha*I_t - Ca_diff/tau_Ca)

Structure exploited (beyond the scan formulation of the predecessor):
  * Ca is a per-lane exponential moving average of u_t = dt*alpha*I_t +
    dt/tau_Ca*mu with per-timestep decay C1 = c1^4 in [0.19, 0.45]; it
    forgets its past in ~8 steps, so Ca'_t is computed (to ~1e-3) as an
    8-tap convolution of u -- a chain of 10 diagonal-weight matmuls on
    the otherwise-idle PE accumulating in PSUM.  The initial condition is
    injected exactly as a virtual u_{-1} = Ca'_0 column.
  * The substep sigmoid arguments comb_k = g_k*cap0 + u_t are 2 more
    diag-matmuls each; the ACT sigmoid reads the PSUM bank directly.
  * The Horner combination sacc = sum_k e1^{3-k} sr_k is 4 diag-matmuls
    with strided rhs (k-slices of sr), also on PE.
  * R keeps the DVE tensor_tensor_scan at substep granularity (data-
    dependent coefficients admit no cheaper form); Pt/Qt are DVE 4x-mode
    tensor_scalar ops from the packed bf16 sig tile (Qt partially on ACT
    for balance).
  * Engine balance: ACT sigmoids + affine I-transforms; PE convolutions;
    DVE scan/TS/racc + most sr products; GPSIMD vsig + a share of sr.

Sharding: batch 32 -> 4 samples per core (pure data parallel). Per core the
4*512 = 2048 lanes are processed as 16 lane-batches of 128 partitions; time is
blocked at TB timesteps with scan-state carried across blocks.

Host side does all parameter transforms (including the diagonal PE weight
matrices) and the (b,t,h) <-> (lane, t) transposes in numpy.
"""

import numpy as np
from contextlib import ExitStack

import concourse.bass as bass
import concourse.mybir as mybir
import concourse.tile as tile
from concourse.bass_utils import run_bass_kernel_spmd

f32 = mybir.dt.float32
bf16 = mybir.dt.bfloat16
AF = mybir.ActivationFunctionType
OP = mybir.AluOpType

B, T, H = 32, 2048, 512
K = 4               # ode substeps per timestep
NCORES = 8
BPC = B // NCORES   # batch per core (4)
GH = H // 128       # h-groups of 128 (4)
NLB = BPC * GH      # lane batches per core (16)
PD = 128            # partitions
TB = 512            # timesteps per block
J = 8               # Ca convolution taps (C1^J <= 0.45^8 ~ 2e-3)
NPAR = 20
NW = 19             # diag weight slots per h-group:
                    #   0..9   conv taps diag(C1^j)
                    #   10..11 diag(SC1*G1), diag(SC2*G2); 12 diag(G3-1)
                    #   13..16 sacc diag(e1^{3-k}), slot 16 = identity
                    #   17..18 diag(SC1), diag(SC2)
SRGP = 0            # sr instances (of 20) steered to GPSIMD
QTACT = 0           # Qt instances (of 20) steered to ACT
RACCACT = 20        # racc instances (of 20) routed via ACT sacc-evacuation
RACCGP = 18         # of the ACT-evacuated raccs, how many go to GPSIMD
VSFILL = 2          # first units whose vsig runs on the (idle) DVE
UVWDVE = 1          # first lbs whose u/V/W2 run on the (idle) DVE
EVACDVE = 2         # first units whose cap0s evacuation runs on the DVE

(C1, G1, G2, G3, SC0, SC1, SC2, SC3, BIAS, UC,
 UA, SV, AV, CP, QM, QA, SW2, E1, E14, CA0) = range(NPAR)


def build_program(Tn=T, tb=TB, nlb=NLB, n_devices=NCORES):
    """Build the Bass program (SPMD; same program on every core)."""
    nblk = Tn // tb
    S = K * tb
    nc = bass.Bass("TRN2", target_bir_lowering=False, debug=False,
                   num_devices=n_devices)
    I_d = nc.dram_tensor("i_ca", [nlb, PD, Tn], f32, kind="ExternalInput").ap()
    par_d = nc.dram_tensor("par", [PD, nlb * NPAR], f32,
                           kind="ExternalInput").ap()
    dw_d = nc.dram_tensor("dw", [GH, PD, NW * PD], bf16,
                          kind="ExternalInput").ap()
    O_d = [[nc.dram_tensor(f"epsc_{lb}_{blk}", [PD, tb], f32,
                           kind="ExternalOutput").ap()
            for blk in range(nblk)] for lb in range(nlb)]

    with ExitStack() as ctx:
        tc = ctx.enter_context(tile.TileContext(nc))
        vpool = ctx.enter_context(tc.tile_pool(name="vhand", bufs=2))
        wpool = ctx.enter_context(tc.tile_pool(name="w2hand", bufs=3))
        spool = ctx.enter_context(tc.tile_pool(name="sig", bufs=7))
        cspool = ctx.enter_context(tc.tile_pool(name="cap0s", bufs=2))
        bpool = ctx.enter_context(tc.tile_pool(name="bshort", bufs=4))
        cpool = ctx.enter_context(tc.tile_pool(name="bcarry", bufs=4))
        rpool = ctx.enter_context(tc.tile_pool(name="rbig", bufs=2))
        ipool = ctx.enter_context(tc.tile_pool(name="inp", bufs=2))
        upool = ctx.enter_context(tc.tile_pool(name="ufull", bufs=2))
        ppool = ctx.enter_context(tc.tile_pool(name="par", bufs=1))
        ps1 = ctx.enter_context(tc.tile_pool(name="ps1", bufs=3, space="PSUM"))
        ps2 = ctx.enter_context(tc.tile_pool(name="ps2", bufs=2, space="PSUM"))
        ps3 = ctx.enter_context(tc.tile_pool(name="ps3", bufs=1, space="PSUM"))

        par = ppool.tile([PD, nlb * NPAR], f32, tag="par")
        nc.sync.dma_start(par[:], par_d)
        dw = ppool.tile([PD, GH * NW * PD], bf16, tag="dw")
        for g in range(GH):
            nc.sync.dma_start(dw[:, g * NW * PD:(g + 1) * NW * PD], dw_d[g])

        def wmat(g, i):
            return dw[:, (g * NW + i) * PD:(g * NW + i + 1) * PD]

        itile_lbs = {}
        ufull_lbs = {}
        vw_lbs = {}
        prev_rb = {}
        prev_e = {}

        def pcol_of(lb):
            return lambda i: par[:, lb * NPAR + i:lb * NPAR + i + 1]

        def fetch_itile(lb, split=False):
            itile_lb = ipool.tile([PD, Tn], f32, tag="itile")
            if split:
                Q4 = Tn // 4
                for q in range(4):
                    nc.sync.dma_start(itile_lb[:, q * Q4:(q + 1) * Q4],
                                      I_d[lb][:, q * Q4:(q + 1) * Q4])
            else:
                nc.sync.dma_start(itile_lb[:], I_d[lb])
            itile_lbs[lb] = itile_lb

        def stage_a0(lb, blk):
            """Half-lb u/V/W2 ACT transforms, spread over the lb's 4 steps
            (u first: the PE conv consumes it immediately; V by a4, W2 by
            bh).  Prefetches the next lb's itile at blk 0."""
            pcol = pcol_of(lb)
            t0 = blk * tb
            HT = Tn // 2
            if blk == 0:
                if lb == 0:
                    fetch_itile(0, split=True)
                if lb + 1 < nlb:
                    fetch_itile(lb + 1)
                uf = upool.tile([PD, J + Tn], bf16, tag="ufull")
                nc.vector.memset(uf[:, 0:J - 1], 0.0)
                nc.vector.tensor_copy(uf[:, J - 1:J], pcol(CA0))
                ufull_lbs[lb] = uf
                Vt = vpool.tile([PD, Tn], bf16, tag="V")
                W2t = wpool.tile([PD, Tn], bf16, tag="W2")
                vw_lbs[lb] = (Vt, W2t)
            itile = itile_lbs[lb]
            uf = ufull_lbs[lb]
            V, W2 = vw_lbs[lb]
            half = lambda h: slice(h * HT, (h + 1) * HT)

            def aff(out, in_, ci, ca, chunk=False):
                if lb < UVWDVE:
                    if chunk:
                        nc.vector.tensor_scalar(out[:, 0:tb], in_[:, 0:tb],
                                                pcol(ci), pcol(ca),
                                                OP.mult, OP.add)
                        nc.vector.tensor_scalar(out[:, tb:], in_[:, tb:],
                                                pcol(ci), pcol(ca),
                                                OP.mult, OP.add)
                    else:
                        nc.vector.tensor_scalar(out, in_, pcol(ci), pcol(ca),
                                                OP.mult, OP.add)
                else:
                    nc.scalar.activation(out, in_, AF.Identity,
                                         bias=pcol(ca), scale=pcol(ci))

            def w2p(out, in_):
                if lb < UVWDVE:
                    nc.vector.tensor_scalar(out, in_, pcol(SW2), 0.0,
                                            OP.mult, OP.add)
                else:
                    nc.scalar.activation(out, in_, AF.Copy, scale=pcol(SW2))

            if blk == 0:
                aff(uf[:, J:J + HT], itile[:, half(0)], UC, UA, chunk=True)
            elif blk == 1:
                aff(uf[:, J + HT:J + Tn], itile[:, half(1)], UC, UA)
                aff(V[:, half(0)], itile[:, half(0)], SV, AV)
            elif blk == 2:
                aff(V[:, half(1)], itile[:, half(1)], SV, AV)
                w2p(W2[:, half(0)], itile[:, half(0)])
            else:
                w2p(W2[:, half(1)], itile[:, half(1)])
            return V[:, t0:t0 + tb], W2[:, t0:t0 + tb]

        def stage_a1(lb, blk, V, W2):
            """cap0 PE conv."""
            g = lb % GH
            t0 = blk * tb
            uf = ufull_lbs[lb]
            # cap0_t = sum_{j=1..J} C1^{j-1} u_{t-j}  (virtual u_{-1}=Ca'_0)
            cap0 = ps1.tile([PD, tb], f32, tag="cap0")
            for j in range(1, J + 1):
                nc.tensor.matmul(cap0[:], wmat(g, j - 1),
                                 uf[:, J + t0 - j:J + t0 - j + tb],
                                 start=(j == 1), stop=(j == J))
            return cap0, V, W2

        def stage_a2(lb, blk, cap0, V, W2):
            """sig0 + cap0 evacuation (ACT), comb PE matmuls."""
            pcol = pcol_of(lb)
            g = lb % GH
            t0 = blk * tb
            sig = spool.tile([PD, S], bf16, tag="sig")
            sig3 = sig[:].rearrange("p (t k) -> p t k", k=K)
            nc.scalar.activation(sig3[:, :, 0], cap0[:], AF.Sigmoid,
                                 bias=pcol(BIAS), scale=pcol(SC0))
            u_i = blk + nblk * lb
            cap0s = cspool.tile([PD, tb], bf16, tag="cap0s")
            if u_i < EVACDVE:
                nc.vector.tensor_copy(cap0s[:], cap0[:])
            else:
                nc.scalar.activation(cap0s[:], cap0[:], AF.Copy)

            comb = ps2.tile([PD, 2, tb], f32, tag="comb")
            uslice = ufull_lbs[lb][:, J + t0:J + t0 + tb]
            for k in (1, 2):
                nc.tensor.matmul(comb[:, k - 1], wmat(g, 9 + k), cap0s[:],
                                 start=True, stop=False)
                nc.tensor.matmul(comb[:, k - 1], wmat(g, 16 + k), uslice,
                                 start=False, stop=True)
            # comb3 accumulates in place on top of cap0 (slot 12 = G3-1):
            # cap0 + (g3-1)*cap0s + u = g3*cap0 + u (after sig0/evac reads)
            nc.tensor.matmul(cap0[:], wmat(g, 12), cap0s[:],
                             start=False, stop=False, skip_group_check=True)
            nc.tensor.matmul(cap0[:], wmat(g, 16), uslice,
                             start=False, stop=True, skip_group_check=True)
            return sig, sig3, comb, cap0, V, W2

        def stage_a3(lb, blk, sig, sig3, comb, comb3, V, W2):
            """sig1..3 (ACT, all from PSUM; SC1/SC2 folded into PE wts)."""
            pcol = pcol_of(lb)
            nc.scalar.activation(sig3[:, :, 1:3],
                                 comb[:].rearrange("p k t -> p t k"),
                                 AF.Sigmoid, bias=pcol(BIAS), scale=1.0)
            nc.scalar.activation(sig3[:, :, 3], comb3[:], AF.Sigmoid,
                                 bias=pcol(BIAS), scale=pcol(SC3))
            return sig, sig3, V, W2

        def stage_a4(lb, blk, sig, sig3, V, W2):
            """vsig (GPSIMD; DVE for the first units while the b-pipeline
            is still filling and the DVE is idle)."""
            u = blk + nblk * lb
            vsig = bpool.tile([PD, S], bf16, tag="vsig")
            veng = nc.vector if u < VSFILL else nc.gpsimd
            veng.tensor_mul(
                vsig[:].rearrange("p (t k) -> p t k", k=K),
                sig[:].rearrange("p (t k) -> p t k", k=K),
                V[:].unsqueeze(2).broadcast_to((PD, tb, K)))
            return sig, vsig, W2

        def stage_b1(lb, blk, sig, vsig, W2):
            """Pt, Qt, R scan, sr."""
            pcol = pcol_of(lb)
            u = blk + nblk * lb

            Pt = bpool.tile([PD, S], bf16, tag="Pt")
            nc.vector.tensor_scalar(Pt[:], vsig[:], -1.0, pcol(CP),
                                    OP.mult, OP.add)
            Qt = bpool.tile([PD, S], bf16, tag="Qt")
            if u % 20 < QTACT:
                nc.scalar.activation(Qt[:], sig[:], AF.Identity,
                                     bias=pcol(QA), scale=pcol(QM))
            else:
                nc.vector.tensor_scalar(Qt[:], sig[:], pcol(QM), pcol(QA),
                                        OP.mult, OP.add)

            if blk == 0:
                rbig = rpool.tile([PD, nblk * S + 1], bf16, tag="rbig")
                nc.vector.memset(rbig[:, 0:1], 1.0)
                prev_rb[lb] = rbig
            rbig = prev_rb[lb]
            s0 = blk * S
            nc.vector.tensor_tensor_scan(rbig[:, s0 + 1:s0 + S + 1], Pt[:],
                                         Qt[:], rbig[:, s0:s0 + 1],
                                         OP.mult, OP.add)
            return sig, rbig[:, s0:s0 + S], W2

        def stage_b1x(lb, blk, sig, rsh, W2):
            """sr product, one step after the R scan completes."""
            u = blk + nblk * lb
            sr = bpool.tile([PD, S], bf16, tag="sr")
            seng = nc.gpsimd if u % 20 < SRGP else nc.vector
            seng.tensor_mul(sr[:], sig[:], rsh)
            return sr, W2

        def stage_bh(lb, blk, sr, W2):
            """sacc = sum_k e1^{3-k} sr_k on PE; racc = W2*sacc on DVE."""
            g = lb % GH
            u = blk + nblk * lb
            sacc = ps3.tile([PD, tb], f32, tag="sacc")
            srk = sr[:].rearrange("p (t k) -> p t k", k=K)
            for k in range(K):
                nc.tensor.matmul(sacc[:], wmat(g, 13 + k), srk[:, :, k],
                                 start=(k == 0), stop=(k == K - 1))
            racc = cpool.tile([PD, tb], bf16, tag="racc")
            if u % 20 < RACCACT:
                saccs = cpool.tile([PD, tb], bf16, tag="saccs")
                nc.scalar.activation(saccs[:], sacc[:], AF.Copy)
                reng = nc.gpsimd if u % 20 < RACCGP else nc.vector
                reng.tensor_tensor(racc[:], W2[:], saccs[:], OP.mult)
            else:
                nc.vector.tensor_tensor(racc[:], W2[:], sacc[:], OP.mult)
            return racc

        def stage_b2(lb, blk, racc):
            """EPSC scan + out-DMA."""
            pcol = pcol_of(lb)
            etile = cpool.tile([PD, tb], f32, tag="etile")
            einit = 0.0 if blk == 0 else prev_e[lb][:, tb - 1:tb]
            nc.vector.tensor_tensor_scan(
                etile[:], pcol(E14).to_broadcast((PD, tb)), racc[:],
                einit, OP.mult, OP.add)
            prev_e[lb] = etile
            nc.sync.dma_start(O_d[lb][blk][:], etile[:])

        # software pipeline; emission order per step is chosen so that in
        # each engine queue the instructions that free PSUM banks / unblock
        # the PE come first (ACT has no execution-queue lookahead):
        #   a2(i-2): sig0+evac [ACT], comb [PE]
        #   a3(i-3): sig123 [ACT], vsig [GP]
        #   a0(i):   u/V/W2 [ACT]
        #   a1(i-1): cap0 conv [PE]
        #   b1(i-5): Pt/Qt/scan/sr [DVE/GP]
        #   bh(i-7): sacc [PE], racc [DVE]
        #   b2(i-9): EPSC scan + DMA [DVE]
        units = [(lb, blk) for lb in range(nlb) for blk in range(nblk)]
        n = len(units)
        a0o, a1o, a2o, a3o, a4o, b1o, b1xo, bho = ({}, {}, {}, {}, {}, {},
                                                   {}, {})
        for i in range(n + 9):
            if 2 <= i <= n + 1:
                a2o[i - 2] = stage_a2(*units[i - 2], *a1o.pop(i - 2))
            if 3 <= i <= n + 2:
                a3o[i - 3] = stage_a3(*units[i - 3], *a2o.pop(i - 3))
            if i < n:
                a0o[i] = stage_a0(*units[i])
            if 1 <= i <= n:
                a1o[i - 1] = stage_a1(*units[i - 1], *a0o.pop(i - 1))
            if 4 <= i <= n + 3:
                a4o[i - 4] = stage_a4(*units[i - 4], *a3o.pop(i - 4))
            if 5 <= i <= n + 4:
                b1o[i - 5] = stage_b1(*units[i - 5], *a4o.pop(i - 5))
            if 6 <= i <= n + 5:
                b1xo[i - 6] = stage_b1x(*units[i - 6], *b1o.pop(i - 6))
            if 7 <= i <= n + 6:
                bho[i - 7] = stage_bh(*units[i - 7], *b1xo.pop(i - 7))
            if i >= 9:
                stage_b2(*units[i - 9], bho.pop(i - 9))

    import bass_rust
    bass_rust.generate_event_semaphores(nc)
    return nc


def derive_params(log_Ca_mu, log_Ca_sigma, log_tau_Ca, log_alpha, log_tau_EPSC,
                  log_beta, presigmoid_P_rel_max, log_k_recov_min,
                  log_k_recov_delta, ode_steps):
    """Host-side param math (fp64): ([H, NPAR] fp32, [GH, PD, NW*PD] fp32)."""
    d = np.float64
    dt = 1.0 / int(ode_steps)
    mu = np.exp(log_Ca_mu.astype(d))
    sigma = np.exp(log_Ca_sigma.astype(d))
    tau_Ca = np.exp(log_tau_Ca.astype(d))
    alpha = np.exp(log_alpha.astype(d))
    tau_E = np.exp(log_tau_EPSC.astype(d))
    beta = np.exp(log_beta.astype(d))
    Prm = 1.0 / (1.0 + np.exp(-presigmoid_P_rel_max.astype(d)))
    k_min = np.exp(log_k_recov_min.astype(d))
    k_delta = np.exp(log_k_recov_delta.astype(d))

    c1 = 1.0 - dt / tau_Ca
    S1 = np.ones_like(c1)
    S2 = 1.0 + c1
    S3 = 1.0 + c1 + c1 ** 2
    S4 = S3 + c1 ** 3
    e1 = 1.0 - dt / tau_E

    n = log_Ca_mu.shape[0]
    par = np.zeros((n, NPAR), np.float64)
    par[:, C1] = c1 ** 4
    par[:, G1] = c1 * S4 / S1
    par[:, G2] = c1 ** 2 * S4 / S2
    par[:, G3] = c1 ** 3 * S4 / S3
    par[:, SC0] = S4 / sigma
    par[:, SC1] = S1 / sigma
    par[:, SC2] = S2 / sigma
    par[:, SC3] = S3 / sigma
    par[:, BIAS] = -mu / sigma
    par[:, UC] = dt * alpha
    par[:, UA] = dt / tau_Ca * mu
    par[:, SV] = dt * Prm
    par[:, AV] = dt * k_delta
    par[:, CP] = 1.0 - dt * k_min
    par[:, QM] = dt * k_delta
    par[:, QA] = dt * k_min
    par[:, SW2] = -dt * beta * Prm
    par[:, E1] = e1
    par[:, E14] = e1 ** 4
    par[:, CA0] = mu / S4

    C1v = par[:, C1].reshape(GH, PD)
    Gv = np.stack([par[:, G1], par[:, G2], par[:, G3]], -1).reshape(GH, PD, 3)
    e1v = e1.reshape(GH, PD)
    dwm = np.zeros((GH, PD, NW * PD), np.float64)
    idx = np.arange(PD)
    for g in range(GH):
        for j in range(J):
            dwm[g, idx, j * PD + idx] = C1v[g] ** j          # slots 0..9
        SCv = np.stack([par[:, SC1], par[:, SC2]], -1).reshape(GH, PD, 2)
        for k in range(2):
            dwm[g, idx, (10 + k) * PD + idx] = SCv[g, :, k] * Gv[g, :, k]
            dwm[g, idx, (17 + k) * PD + idx] = SCv[g, :, k]
        dwm[g, idx, 12 * PD + idx] = Gv[g, :, 2] - 1.0       # slot 12: G3-1
        for k in range(K):
            dwm[g, idx, (13 + k) * PD + idx] = e1v[g] ** (3 - k)  # 13..16
    return par.astype(np.float32), dwm.astype(np.float32)


_PROG = None
LAST_RESULTS = None  # BassKernelResults of the most recent kernel() call


def _get_program():
    global _PROG
    if _PROG is None:
        _PROG = build_program()
    return _PROG


def _to_bf16(x):
    """fp32 -> bfloat16 numpy array (ml_dtypes)."""
    import ml_dtypes
    return x.astype(ml_dtypes.bfloat16)


def kernel(I_Ca, log_Ca_mu, log_Ca_sigma, log_tau_Ca, log_alpha, log_tau_EPSC,
           log_beta, presigmoid_P_rel_max, log_k_recov_min, log_k_recov_delta,
           ode_steps):
    assert int(ode_steps) == K, f"kernel hardcodes {K} substeps"
    I_Ca = np.asarray(I_Ca, np.float32)
    assert I_Ca.shape == (B, T, H)

    par_h, dwm = derive_params(
        np.asarray(log_Ca_mu), np.asarray(log_Ca_sigma), np.asarray(log_tau_Ca),
        np.asarray(log_alpha), np.asarray(log_tau_EPSC), np.asarray(log_beta),
        np.asarray(presigmoid_P_rel_max), np.asarray(log_k_recov_min),
        np.asarray(log_k_recov_delta), ode_steps)

    # lane-batch lb = b_local*GH + g holds lanes h = g*128 + p
    par_lb = par_h.reshape(GH, PD, NPAR)
    par_core = np.ascontiguousarray(
        np.broadcast_to(par_lb[None], (BPC, GH, PD, NPAR)).reshape(
            NLB, PD, NPAR).transpose(1, 0, 2).reshape(PD, NLB * NPAR))
    dw_core = _to_bf16(np.ascontiguousarray(dwm))

    nc = _get_program()
    in_maps = []
    for c in range(NCORES):
        Ic = I_Ca[c * BPC:(c + 1) * BPC]                    # [BPC, T, H]
        Ic = Ic.reshape(BPC, T, GH, PD).transpose(0, 2, 3, 1)
        in_maps.append({
            "i_ca": np.ascontiguousarray(Ic.reshape(NLB, PD, T)),
            "par": par_core,
            "dw": dw_core,
        })

    res = run_bass_kernel_spmd(nc, in_maps, core_ids=list(range(NCORES)))
    global LAST_RESULTS
    LAST_RESULTS = res
    nblk = T // TB
    out = np.empty((B, T, H), np.float32)
    for c in range(NCORES):
        Oc = np.stack([
            np.concatenate([res.results[c][f"epsc_{lb}_{blk}"]
                            for blk in range(nblk)], axis=1)
            for lb in range(NLB)])                          # [NLB, PD, T]
        Oc = Oc.reshape(BPC, GH, PD, T)
        out[c * BPC:(c + 1) * BPC] = Oc.transpose(0, 3, 1, 2).reshape(BPC, T, H)
    return out


# revision 53
# speedup vs baseline: 1.0289x; 1.0111x over previous
"""Trainium2 Bass kernel for the FD (facilitation-depression) synapse layer.

Reference computes, per (b, h) lane, a sequential recurrence over T timesteps
with K=4 unrolled Euler substeps:

    Ca_diff = Ca - Ca_mu
    sig     = sigmoid(Ca_diff / Ca_sigma)
    temp    = P_rel_max*sig * R * I_t
    EPSC    = EPSC - dt*(EPSC / tau_EPSC + beta*temp)
    R       = R + dt*((k_min + k_delta*sig)*(1-R) - temp)
    Ca      = Ca + dt*(alpha*I_t - Ca_diff/tau_Ca)

Structure exploited (beyond the scan formulation of the predecessor):
  * Ca is a per-lane exponential moving average of u_t = dt*alpha*I_t +
    dt/tau_Ca*mu with per-timestep decay C1 = c1^4 in [0.19, 0.45]; it
    forgets its past in ~8 steps, so Ca'_t is computed (to ~1e-3) as an
    8-tap convolution of u -- a chain of 10 diagonal-weight matmuls on
    the otherwise-idle PE accumulating in PSUM.  The initial condition is
    injected exactly as a virtual u_{-1} = Ca'_0 column.
  * The substep sigmoid arguments comb_k = g_k*cap0 + u_t are 2 more
    diag-matmuls each; the ACT sigmoid reads the PSUM bank directly.
  * The Horner combination sacc = sum_k e1^{3-k} sr_k is 4 diag-matmuls
    with strided rhs (k-slices of sr), also on PE.
  * R keeps the DVE tensor_tensor_scan at substep granularity (data-
    dependent coefficients admit no cheaper form); Pt/Qt are DVE 4x-mode
    tensor_scalar ops from the packed bf16 sig tile (Qt partially on ACT
    for balance).
  * Engine balance: ACT sigmoids + affine I-transforms; PE convolutions;
    DVE scan/TS/racc + most sr products; GPSIMD vsig + a share of sr.

Sharding: batch 32 -> 4 samples per core (pure data parallel). Per core the
4*512 = 2048 lanes are processed as 16 lane-batches of 128 partitions; time is
blocked at TB timesteps with scan-state carried across blocks.

Host side does all parameter transforms (including the diagonal PE weight
matrices) and the (b,t,h) <-> (lane, t) transposes in numpy.
"""

import numpy as np
from contextlib import ExitStack

import concourse.bass as bass
import concourse.mybir as mybir
import concourse.tile as tile
from concourse.bass_utils import run_bass_kernel_spmd

f32 = mybir.dt.float32
bf16 = mybir.dt.bfloat16
AF = mybir.ActivationFunctionType
OP = mybir.AluOpType

B, T, H = 32, 2048, 512
K = 4               # ode substeps per timestep
NCORES = 8
BPC = B // NCORES   # batch per core (4)
GH = H // 128       # h-groups of 128 (4)
NLB = BPC * GH      # lane batches per core (16)
PD = 128            # partitions
TB = 512            # timesteps per block
J = 8               # Ca convolution taps (C1^J <= 0.45^8 ~ 2e-3)
NPAR = 20
NW = 19             # diag weight slots per h-group:
                    #   0..9   conv taps diag(C1^j)
                    #   10..11 diag(SC1*G1), diag(SC2*G2); 12 diag(G3-1)
                    #   13..16 sacc diag(e1^{3-k}), slot 16 = identity
                    #   17..18 diag(SC1), diag(SC2)
SRGP = 0            # sr instances (of 20) steered to GPSIMD
QTACT = 0           # Qt instances (of 20) steered to ACT
RACCACT = 20        # racc instances (of 20) routed via ACT sacc-evacuation
RACCGP = 18         # of the ACT-evacuated raccs, how many go to GPSIMD
VSFILL = 2          # first units whose vsig runs on the (idle) DVE
UVWDVE = 1          # first lbs whose u/V/W2 run on the (idle) DVE
EVACDVE = 2         # first units whose cap0s evacuation runs on the DVE

(C1, G1, G2, G3, SC0, SC1, SC2, SC3, BIAS, UC,
 UA, SV, AV, CP, QM, QA, SW2, E1, E14, CA0) = range(NPAR)


def build_program(Tn=T, tb=TB, nlb=NLB, n_devices=NCORES):
    """Build the Bass program (SPMD; same program on every core)."""
    nblk = Tn // tb
    S = K * tb
    nc = bass.Bass("TRN2", target_bir_lowering=False, debug=False,
                   num_devices=n_devices)
    I_d = nc.dram_tensor("i_ca", [nlb, PD, Tn], f32, kind="ExternalInput").ap()
    par_d = nc.dram_tensor("par", [PD, nlb * NPAR], f32,
                           kind="ExternalInput").ap()
    dw_d = nc.dram_tensor("dw", [GH, PD, NW * PD], bf16,
                          kind="ExternalInput").ap()
    O_d = [[nc.dram_tensor(f"epsc_{lb}_{blk}", [PD, tb], f32,
                           kind="ExternalOutput").ap()
            for blk in range(nblk)] for lb in range(nlb)]

    with ExitStack() as ctx:
        tc = ctx.enter_context(tile.TileContext(nc))
        vpool = ctx.enter_context(tc.tile_pool(name="vhand", bufs=2))
        wpool = ctx.enter_context(tc.tile_pool(name="w2hand", bufs=3))
        spool = ctx.enter_context(tc.tile_pool(name="sig", bufs=7))
        cspool = ctx.enter_context(tc.tile_pool(name="cap0s", bufs=2))
        bpool = ctx.enter_context(tc.tile_pool(name="bshort", bufs=4))
        cpool = ctx.enter_context(tc.tile_pool(name="bcarry", bufs=4))
        rpool = ctx.enter_context(tc.tile_pool(name="rbig", bufs=2))
        ipool = ctx.enter_context(tc.tile_pool(name="inp", bufs=2))
        upool = ctx.enter_context(tc.tile_pool(name="ufull", bufs=2))
        ppool = ctx.enter_context(tc.tile_pool(name="par", bufs=1))
        ps1 = ctx.enter_context(tc.tile_pool(name="ps1", bufs=3, space="PSUM"))
        ps2 = ctx.enter_context(tc.tile_pool(name="ps2", bufs=2, space="PSUM"))
        ps3 = ctx.enter_context(tc.tile_pool(name="ps3", bufs=1, space="PSUM"))

        par = ppool.tile([PD, nlb * NPAR], f32, tag="par")
        nc.sync.dma_start(par[:], par_d)
        dw = ppool.tile([PD, GH * NW * PD], bf16, tag="dw")

        def wmat(g, i):
            return dw[:, (g * NW + i) * PD:(g * NW + i + 1) * PD]

        itile_lbs = {}
        ufull_lbs = {}
        _dma_plan_done = []
        vw_lbs = {}
        prev_rb = {}
        prev_e = {}

        def pcol_of(lb):
            return lambda i: par[:, lb * NPAR + i:lb * NPAR + i + 1]

        def fetch_itile(lb, split=False):
            itile_lb = ipool.tile([PD, Tn], f32, tag="itile")
            if split:
                Q4 = Tn // 4
                for q in range(4):
                    nc.sync.dma_start(itile_lb[:, q * Q4:(q + 1) * Q4],
                                      I_d[lb][:, q * Q4:(q + 1) * Q4])
            else:
                nc.sync.dma_start(itile_lb[:], I_d[lb])
            itile_lbs[lb] = itile_lb

        def stage_a0(lb, blk):
            """Half-lb u/V/W2 ACT transforms, spread over the lb's 4 steps
            (u first: the PE conv consumes it immediately; V by a4, W2 by
            bh).  Prefetches the next lb's itile at blk 0."""
            pcol = pcol_of(lb)
            t0 = blk * tb
            HT = Tn // 2
            if blk == 0:
                if lb == 0:
                    # startup DMA order: first input quarter, then the g=0
                    # weights the first conv needs, then the rest
                    itile_lb = ipool.tile([PD, Tn], f32, tag="itile")
                    Q4 = Tn // 4
                    nc.sync.dma_start(itile_lb[:, 0:Q4], I_d[0][:, 0:Q4])
                    nc.sync.dma_start(dw[:, 0:NW * PD], dw_d[0])
                    for q in range(1, 4):
                        nc.sync.dma_start(itile_lb[:, q * Q4:(q + 1) * Q4],
                                          I_d[0][:, q * Q4:(q + 1) * Q4])
                    for g_ in range(1, GH):
                        nc.sync.dma_start(
                            dw[:, g_ * NW * PD:(g_ + 1) * NW * PD],
                            dw_d[g_])
                    itile_lbs[0] = itile_lb
                if lb + 1 < nlb:
                    fetch_itile(lb + 1)
                uf = upool.tile([PD, J + Tn], bf16, tag="ufull")
                nc.vector.memset(uf[:, 0:J - 1], 0.0)
                nc.vector.tensor_copy(uf[:, J - 1:J], pcol(CA0))
                ufull_lbs[lb] = uf
                Vt = vpool.tile([PD, Tn], bf16, tag="V")
                W2t = wpool.tile([PD, Tn], bf16, tag="W2")
                vw_lbs[lb] = (Vt, W2t)
            itile = itile_lbs[lb]
            uf = ufull_lbs[lb]
            V, W2 = vw_lbs[lb]
            half = lambda h: slice(h * HT, (h + 1) * HT)

            def aff(out, in_, ci, ca, chunk=False):
                if lb < UVWDVE:
                    if chunk:
                        nc.vector.tensor_scalar(out[:, 0:tb], in_[:, 0:tb],
                                                pcol(ci), pcol(ca),
                                                OP.mult, OP.add)
                        nc.vector.tensor_scalar(out[:, tb:], in_[:, tb:],
                                                pcol(ci), pcol(ca),
                                                OP.mult, OP.add)
                    else:
                        nc.vector.tensor_scalar(out, in_, pcol(ci), pcol(ca),
                                                OP.mult, OP.add)
                else:
                    nc.scalar.activation(out, in_, AF.Identity,
                                         bias=pcol(ca), scale=pcol(ci))

            def w2p(out, in_):
                if lb < UVWDVE:
                    nc.vector.tensor_scalar(out, in_, pcol(SW2), 0.0,
                                            OP.mult, OP.add)
                else:
                    nc.scalar.activation(out, in_, AF.Copy, scale=pcol(SW2))

            if blk == 0:
                aff(uf[:, J:J + HT], itile[:, half(0)], UC, UA, chunk=True)
            elif blk == 1:
                aff(uf[:, J + HT:J + Tn], itile[:, half(1)], UC, UA)
                aff(V[:, half(0)], itile[:, half(0)], SV, AV)
            elif blk == 2:
                aff(V[:, half(1)], itile[:, half(1)], SV, AV)
                w2p(W2[:, half(0)], itile[:, half(0)])
            else:
                w2p(W2[:, half(1)], itile[:, half(1)])
            return V[:, t0:t0 + tb], W2[:, t0:t0 + tb]

        def stage_a1(lb, blk, V, W2):
            """cap0 PE conv."""
            g = lb % GH
            t0 = blk * tb
            uf = ufull_lbs[lb]
            # cap0_t = sum_{j=1..J} C1^{j-1} u_{t-j}  (virtual u_{-1}=Ca'_0)
            cap0 = ps1.tile([PD, tb], f32, tag="cap0")
            for j in range(1, J + 1):
                nc.tensor.matmul(cap0[:], wmat(g, j - 1),
                                 uf[:, J + t0 - j:J + t0 - j + tb],
                                 start=(j == 1), stop=(j == J))
            return cap0, V, W2

        def stage_a2(lb, blk, cap0, V, W2):
            """sig0 + cap0 evacuation (ACT), comb PE matmuls."""
            pcol = pcol_of(lb)
            g = lb % GH
            t0 = blk * tb
            sig = spool.tile([PD, S], bf16, tag="sig")
            sig3 = sig[:].rearrange("p (t k) -> p t k", k=K)
            nc.scalar.activation(sig3[:, :, 0], cap0[:], AF.Sigmoid,
                                 bias=pcol(BIAS), scale=pcol(SC0))
            u_i = blk + nblk * lb
            cap0s = cspool.tile([PD, tb], bf16, tag="cap0s")
            if u_i < EVACDVE:
                nc.vector.tensor_copy(cap0s[:], cap0[:])
            else:
                nc.scalar.activation(cap0s[:], cap0[:], AF.Copy)

            comb = ps2.tile([PD, 2, tb], f32, tag="comb")
            uslice = ufull_lbs[lb][:, J + t0:J + t0 + tb]
            for k in (1, 2):
                nc.tensor.matmul(comb[:, k - 1], wmat(g, 9 + k), cap0s[:],
                                 start=True, stop=False)
                nc.tensor.matmul(comb[:, k - 1], wmat(g, 16 + k), uslice,
                                 start=False, stop=True)
            # comb3 accumulates in place on top of cap0 (slot 12 = G3-1):
            # cap0 + (g3-1)*cap0s + u = g3*cap0 + u (after sig0/evac reads)
            nc.tensor.matmul(cap0[:], wmat(g, 12), cap0s[:],
                             start=False, stop=False, skip_group_check=True)
            nc.tensor.matmul(cap0[:], wmat(g, 16), uslice,
                             start=False, stop=True, skip_group_check=True)
            return sig, sig3, comb, cap0, V, W2

        def stage_a3(lb, blk, sig, sig3, comb, comb3, V, W2):
            """sig1..3 (ACT, all from PSUM; SC1/SC2 folded into PE wts)."""
            pcol = pcol_of(lb)
            nc.scalar.activation(sig3[:, :, 1:3],
                                 comb[:].rearrange("p k t -> p t k"),
                                 AF.Sigmoid, bias=pcol(BIAS), scale=1.0)
            nc.scalar.activation(sig3[:, :, 3], comb3[:], AF.Sigmoid,
                                 bias=pcol(BIAS), scale=pcol(SC3))
            return sig, sig3, V, W2

        def stage_a4(lb, blk, sig, sig3, V, W2):
            """vsig (GPSIMD; DVE for the first units while the b-pipeline
            is still filling and the DVE is idle)."""
            u = blk + nblk * lb
            vsig = bpool.tile([PD, S], bf16, tag="vsig")
            veng = nc.vector if u < VSFILL else nc.gpsimd
            veng.tensor_mul(
                vsig[:].rearrange("p (t k) -> p t k", k=K),
                sig[:].rearrange("p (t k) -> p t k", k=K),
                V[:].unsqueeze(2).broadcast_to((PD, tb, K)))
            return sig, vsig, W2

        def stage_b1(lb, blk, sig, vsig, W2):
            """Pt, Qt, R scan, sr."""
            pcol = pcol_of(lb)
            u = blk + nblk * lb

            Pt = bpool.tile([PD, S], bf16, tag="Pt")
            nc.vector.tensor_scalar(Pt[:], vsig[:], -1.0, pcol(CP),
                                    OP.mult, OP.add)
            Qt = bpool.tile([PD, S], bf16, tag="Qt")
            if u % 20 < QTACT:
                nc.scalar.activation(Qt[:], sig[:], AF.Identity,
                                     bias=pcol(QA), scale=pcol(QM))
            else:
                nc.vector.tensor_scalar(Qt[:], sig[:], pcol(QM), pcol(QA),
                                        OP.mult, OP.add)

            if blk == 0:
                rbig = rpool.tile([PD, nblk * S + 1], bf16, tag="rbig")
                nc.vector.memset(rbig[:, 0:1], 1.0)
                prev_rb[lb] = rbig
            rbig = prev_rb[lb]
            s0 = blk * S
            nc.vector.tensor_tensor_scan(rbig[:, s0 + 1:s0 + S + 1], Pt[:],
                                         Qt[:], rbig[:, s0:s0 + 1],
                                         OP.mult, OP.add)
            return sig, rbig[:, s0:s0 + S], W2

        def stage_b1x(lb, blk, sig, rsh, W2):
            """sr product, one step after the R scan completes."""
            u = blk + nblk * lb
            sr = bpool.tile([PD, S], bf16, tag="sr")
            seng = nc.gpsimd if u % 20 < SRGP else nc.vector
            seng.tensor_mul(sr[:], sig[:], rsh)
            return sr, W2

        def stage_bh(lb, blk, sr, W2):
            """sacc = sum_k e1^{3-k} sr_k on PE; racc = W2*sacc on DVE."""
            g = lb % GH
            u = blk + nblk * lb
            sacc = ps3.tile([PD, tb], f32, tag="sacc")
            srk = sr[:].rearrange("p (t k) -> p t k", k=K)
            for k in range(K):
                nc.tensor.matmul(sacc[:], wmat(g, 13 + k), srk[:, :, k],
                                 start=(k == 0), stop=(k == K - 1))
            racc = cpool.tile([PD, tb], bf16, tag="racc")
            if u % 20 < RACCACT:
                saccs = cpool.tile([PD, tb], bf16, tag="saccs")
                nc.scalar.activation(saccs[:], sacc[:], AF.Copy)
                reng = nc.gpsimd if u % 20 < RACCGP else nc.vector
                reng.tensor_tensor(racc[:], W2[:], saccs[:], OP.mult)
            else:
                nc.vector.tensor_tensor(racc[:], W2[:], sacc[:], OP.mult)
            return racc

        def stage_b2(lb, blk, racc):
            """EPSC scan + out-DMA."""
            pcol = pcol_of(lb)
            etile = cpool.tile([PD, tb], f32, tag="etile")
            einit = 0.0 if blk == 0 else prev_e[lb][:, tb - 1:tb]
            nc.vector.tensor_tensor_scan(
                etile[:], pcol(E14).to_broadcast((PD, tb)), racc[:],
                einit, OP.mult, OP.add)
            prev_e[lb] = etile
            nc.sync.dma_start(O_d[lb][blk][:], etile[:])

        # software pipeline; emission order per step is chosen so that in
        # each engine queue the instructions that free PSUM banks / unblock
        # the PE come first (ACT has no execution-queue lookahead):
        #   a2(i-2): sig0+evac [ACT], comb [PE]
        #   a3(i-3): sig123 [ACT], vsig [GP]
        #   a0(i):   u/V/W2 [ACT]
        #   a1(i-1): cap0 conv [PE]
        #   b1(i-5): Pt/Qt/scan/sr [DVE/GP]
        #   bh(i-7): sacc [PE], racc [DVE]
        #   b2(i-9): EPSC scan + DMA [DVE]
        units = [(lb, blk) for lb in range(nlb) for blk in range(nblk)]
        n = len(units)
        a0o, a1o, a2o, a3o, a4o, b1o, b1xo, bho = ({}, {}, {}, {}, {}, {},
                                                   {}, {})
        for i in range(n + 9):
            if 2 <= i <= n + 1:
                a2o[i - 2] = stage_a2(*units[i - 2], *a1o.pop(i - 2))
            if 3 <= i <= n + 2:
                a3o[i - 3] = stage_a3(*units[i - 3], *a2o.pop(i - 3))
            if i < n:
                a0o[i] = stage_a0(*units[i])
            if 1 <= i <= n:
                a1o[i - 1] = stage_a1(*units[i - 1], *a0o.pop(i - 1))
            if 4 <= i <= n + 3:
                a4o[i - 4] = stage_a4(*units[i - 4], *a3o.pop(i - 4))
            if 5 <= i <= n + 4:
                b1o[i - 5] = stage_b1(*units[i - 5], *a4o.pop(i - 5))
            if 6 <= i <= n + 5:
                b1xo[i - 6] = stage_b1x(*units[i - 6], *b1o.pop(i - 6))
            if 7 <= i <= n + 6:
                bho[i - 7] = stage_bh(*units[i - 7], *b1xo.pop(i - 7))
            if i >= 9:
                stage_b2(*units[i - 9], bho.pop(i - 9))

    import bass_rust
    bass_rust.generate_event_semaphores(nc)
    return nc


def derive_params(log_Ca_mu, log_Ca_sigma, log_tau_Ca, log_alpha, log_tau_EPSC,
                  log_beta, presigmoid_P_rel_max, log_k_recov_min,
                  log_k_recov_delta, ode_steps):
    """Host-side param math (fp64): ([H, NPAR] fp32, [GH, PD, NW*PD] fp32)."""
    d = np.float64
    dt = 1.0 / int(ode_steps)
    mu = np.exp(log_Ca_mu.astype(d))
    sigma = np.exp(log_Ca_sigma.astype(d))
    tau_Ca = np.exp(log_tau_Ca.astype(d))
    alpha = np.exp(log_alpha.astype(d))
    tau_E = np.exp(log_tau_EPSC.astype(d))
    beta = np.exp(log_beta.astype(d))
    Prm = 1.0 / (1.0 + np.exp(-presigmoid_P_rel_max.astype(d)))
    k_min = np.exp(log_k_recov_min.astype(d))
    k_delta = np.exp(log_k_recov_delta.astype(d))

    c1 = 1.0 - dt / tau_Ca
    S1 = np.ones_like(c1)
    S2 = 1.0 + c1
    S3 = 1.0 + c1 + c1 ** 2
    S4 = S3 + c1 ** 3
    e1 = 1.0 - dt / tau_E

    n = log_Ca_mu.shape[0]
    par = np.zeros((n, NPAR), np.float64)
    par[:, C1] = c1 ** 4
    par[:, G1] = c1 * S4 / S1
    par[:, G2] = c1 ** 2 * S4 / S2
    par[:, G3] = c1 ** 3 * S4 / S3
    par[:, SC0] = S4 / sigma
    par[:, SC1] = S1 / sigma
    par[:, SC2] = S2 / sigma
    par[:, SC3] = S3 / sigma
    par[:, BIAS] = -mu / sigma
    par[:, UC] = dt * alpha
    par[:, UA] = dt / tau_Ca * mu
    par[:, SV] = dt * Prm
    par[:, AV] = dt * k_delta
    par[:, CP] = 1.0 - dt * k_min
    par[:, QM] = dt * k_delta
    par[:, QA] = dt * k_min
    par[:, SW2] = -dt * beta * Prm
    par[:, E1] = e1
    par[:, E14] = e1 ** 4
    par[:, CA0] = mu / S4

    C1v = par[:, C1].reshape(GH, PD)
    Gv = np.stack([par[:, G1], par[:, G2], par[:, G3]], -1).reshape(GH, PD, 3)
    e1v = e1.reshape(GH, PD)
    dwm = np.zeros((GH, PD, NW * PD), np.float64)
    idx = np.arange(PD)
    for g in range(GH):
        for j in range(J):
            dwm[g, idx, j * PD + idx] = C1v[g] ** j          # slots 0..9
        SCv = np.stack([par[:, SC1], par[:, SC2]], -1).reshape(GH, PD, 2)
        for k in range(2):
            dwm[g, idx, (10 + k) * PD + idx] = SCv[g, :, k] * Gv[g, :, k]
            dwm[g, idx, (17 + k) * PD + idx] = SCv[g, :, k]
        dwm[g, idx, 12 * PD + idx] = Gv[g, :, 2] - 1.0       # slot 12: G3-1
        for k in range(K):
            dwm[g, idx, (13 + k) * PD + idx] = e1v[g] ** (3 - k)  # 13..16
    return par.astype(np.float32), dwm.astype(np.float32)


_PROG = None
LAST_RESULTS = None  # BassKernelResults of the most recent kernel() call


def _get_program():
    global _PROG
    if _PROG is None:
        _PROG = build_program()
    return _PROG


def _to_bf16(x):
    """fp32 -> bfloat16 numpy array (ml_dtypes)."""
    import ml_dtypes
    return x.astype(ml_dtypes.bfloat16)


def kernel(I_Ca, log_Ca_mu, log_Ca_sigma, log_tau_Ca, log_alpha, log_tau_EPSC,
           log_beta, presigmoid_P_rel_max, log_k_recov_min, log_k_recov_delta,
           ode_steps):
    assert int(ode_steps) == K, f"kernel hardcodes {K} substeps"
    I_Ca = np.asarray(I_Ca, np.float32)
    assert I_Ca.shape == (B, T, H)

    par_h, dwm = derive_params(
        np.asarray(log_Ca_mu), np.asarray(log_Ca_sigma), np.asarray(log_tau_Ca),
        np.asarray(log_alpha), np.asarray(log_tau_EPSC), np.asarray(log_beta),
        np.asarray(presigmoid_P_rel_max), np.asarray(log_k_recov_min),
        np.asarray(log_k_recov_delta), ode_steps)

    # lane-batch lb = b_local*GH + g holds lanes h = g*128 + p
    par_lb = par_h.reshape(GH, PD, NPAR)
    par_core = np.ascontiguousarray(
        np.broadcast_to(par_lb[None], (BPC, GH, PD, NPAR)).reshape(
            NLB, PD, NPAR).transpose(1, 0, 2).reshape(PD, NLB * NPAR))
    dw_core = _to_bf16(np.ascontiguousarray(dwm))

    nc = _get_program()
    in_maps = []
    for c in range(NCORES):
        Ic = I_Ca[c * BPC:(c + 1) * BPC]                    # [BPC, T, H]
        Ic = Ic.reshape(BPC, T, GH, PD).transpose(0, 2, 3, 1)
        in_maps.append({
            "i_ca": np.ascontiguousarray(Ic.reshape(NLB, PD, T)),
            "par": par_core,
            "dw": dw_core,
        })

    res = run_bass_kernel_spmd(nc, in_maps, core_ids=list(range(NCORES)))
    global LAST_RESULTS
    LAST_RESULTS = res
    nblk = T // TB
    out = np.empty((B, T, H), np.float32)
    for c in range(NCORES):
        Oc = np.stack([
            np.concatenate([res.results[c][f"epsc_{lb}_{blk}"]
                            for blk in range(nblk)], axis=1)
            for lb in range(NLB)])                          # [NLB, PD, T]
        Oc = Oc.reshape(BPC, GH, PD, T)
        out[c * BPC:(c + 1) * BPC] = Oc.transpose(0, 3, 1, 2).reshape(BPC, T, H)
    return out


# revision 54
# speedup vs baseline: 1.0327x; 1.0037x over previous
"""Trainium2 Bass kernel for the FD (facilitation-depression) synapse layer.

Reference computes, per (b, h) lane, a sequential recurrence over T timesteps
with K=4 unrolled Euler substeps:

    Ca_diff = Ca - Ca_mu
    sig     = sigmoid(Ca_diff / Ca_sigma)
    temp    = P_rel_max*sig * R * I_t
    EPSC    = EPSC - dt*(EPSC / tau_EPSC + beta*temp)
    R       = R + dt*((k_min + k_delta*sig)*(1-R) - temp)
    Ca      = Ca + dt*(alpha*I_t - Ca_diff/tau_Ca)

Structure exploited (beyond the scan formulation of the predecessor):
  * Ca is a per-lane exponential moving average of u_t = dt*alpha*I_t +
    dt/tau_Ca*mu with per-timestep decay C1 = c1^4 in [0.19, 0.45]; it
    forgets its past in ~8 steps, so Ca'_t is computed (to ~1e-3) as an
    8-tap convolution of u -- a chain of 10 diagonal-weight matmuls on
    the otherwise-idle PE accumulating in PSUM.  The initial condition is
    injected exactly as a virtual u_{-1} = Ca'_0 column.
  * The substep sigmoid arguments comb_k = g_k*cap0 + u_t are 2 more
    diag-matmuls each; the ACT sigmoid reads the PSUM bank directly.
  * The Horner combination sacc = sum_k e1^{3-k} sr_k is 4 diag-matmuls
    with strided rhs (k-slices of sr), also on PE.
  * R keeps the DVE tensor_tensor_scan at substep granularity (data-
    dependent coefficients admit no cheaper form); Pt/Qt are DVE 4x-mode
    tensor_scalar ops from the packed bf16 sig tile (Qt partially on ACT
    for balance).
  * Engine balance: ACT sigmoids + affine I-transforms; PE convolutions;
    DVE scan/TS/racc + most sr products; GPSIMD vsig + a share of sr.

Sharding: batch 32 -> 4 samples per core (pure data parallel). Per core the
4*512 = 2048 lanes are processed as 16 lane-batches of 128 partitions; time is
blocked at TB timesteps with scan-state carried across blocks.

Host side does all parameter transforms (including the diagonal PE weight
matrices) and the (b,t,h) <-> (lane, t) transposes in numpy.
"""

import numpy as np
from contextlib import ExitStack

import concourse.bass as bass
import concourse.mybir as mybir
import concourse.tile as tile
from concourse.bass_utils import run_bass_kernel_spmd

f32 = mybir.dt.float32
bf16 = mybir.dt.bfloat16
AF = mybir.ActivationFunctionType
OP = mybir.AluOpType

B, T, H = 32, 2048, 512
K = 4               # ode substeps per timestep
NCORES = 8
BPC = B // NCORES   # batch per core (4)
GH = H // 128       # h-groups of 128 (4)
NLB = BPC * GH      # lane batches per core (16)
PD = 128            # partitions
TB = 512            # timesteps per block
J = 8               # Ca convolution taps (C1^J <= 0.45^8 ~ 2e-3)
NPAR = 20
NW = 19             # diag weight slots per h-group:
                    #   0..9   conv taps diag(C1^j)
                    #   10..11 diag(SC1*G1), diag(SC2*G2); 12 diag(G3-1)
                    #   13..16 sacc diag(e1^{3-k}), slot 16 = identity
                    #   17..18 diag(SC1), diag(SC2)
SRGP = 0            # sr instances (of 20) steered to GPSIMD
QTACT = 0           # Qt instances (of 20) steered to ACT
RACCACT = 20        # racc instances (of 20) routed via ACT sacc-evacuation
RACCGP = 18         # of the ACT-evacuated raccs, how many go to GPSIMD
VSFILL = 2          # first units whose vsig runs on the (idle) DVE
UVWDVE = 1          # first lbs whose u/V/W2 run on the (idle) DVE
EVACDVE = 2         # first units whose cap0s evacuation runs on the DVE

(C1, G1, G2, G3, SC0, SC1, SC2, SC3, BIAS, UC,
 UA, SV, AV, CP, QM, QA, SW2, E1, E14, CA0) = range(NPAR)


def build_program(Tn=T, tb=TB, nlb=NLB, n_devices=NCORES):
    """Build the Bass program (SPMD; same program on every core)."""
    nblk = Tn // tb
    S = K * tb
    nc = bass.Bass("TRN2", target_bir_lowering=False, debug=False,
                   num_devices=n_devices)
    I_d = nc.dram_tensor("i_ca", [nlb, PD, Tn], f32, kind="ExternalInput").ap()
    par_d = nc.dram_tensor("par", [PD, nlb * NPAR], f32,
                           kind="ExternalInput").ap()
    dw_d = nc.dram_tensor("dw", [GH, PD, NW * PD], bf16,
                          kind="ExternalInput").ap()
    O_d = [[nc.dram_tensor(f"epsc_{lb}_{blk}", [PD, tb], f32,
                           kind="ExternalOutput").ap()
            for blk in range(nblk)] for lb in range(nlb)]

    with ExitStack() as ctx:
        tc = ctx.enter_context(tile.TileContext(nc))
        vpool = ctx.enter_context(tc.tile_pool(name="vhand", bufs=2))
        wpool = ctx.enter_context(tc.tile_pool(name="w2hand", bufs=3))
        spool = ctx.enter_context(tc.tile_pool(name="sig", bufs=7))
        cspool = ctx.enter_context(tc.tile_pool(name="cap0s", bufs=2))
        bpool = ctx.enter_context(tc.tile_pool(name="bshort", bufs=4))
        cpool = ctx.enter_context(tc.tile_pool(name="bcarry", bufs=4))
        rpool = ctx.enter_context(tc.tile_pool(name="rbig", bufs=2))
        ipool = ctx.enter_context(tc.tile_pool(name="inp", bufs=2))
        upool = ctx.enter_context(tc.tile_pool(name="ufull", bufs=2))
        ppool = ctx.enter_context(tc.tile_pool(name="par", bufs=1))
        ps1 = ctx.enter_context(tc.tile_pool(name="ps1", bufs=3, space="PSUM"))
        ps2 = ctx.enter_context(tc.tile_pool(name="ps2", bufs=2, space="PSUM"))
        ps3 = ctx.enter_context(tc.tile_pool(name="ps3", bufs=1, space="PSUM"))

        par = ppool.tile([PD, nlb * NPAR], f32, tag="par")
        nc.sync.dma_start(par[:], par_d)
        dw = ppool.tile([PD, GH * NW * PD], bf16, tag="dw")

        def wmat(g, i):
            return dw[:, (g * NW + i) * PD:(g * NW + i + 1) * PD]

        itile_lbs = {}
        ufull_lbs = {}
        _dma_plan_done = []
        vw_lbs = {}
        prev_rb = {}
        prev_e = {}

        def pcol_of(lb):
            return lambda i: par[:, lb * NPAR + i:lb * NPAR + i + 1]

        def fetch_itile(lb, split=False):
            itile_lb = ipool.tile([PD, Tn], f32, tag="itile")
            if split:
                Q4 = Tn // 4
                for q in range(4):
                    nc.sync.dma_start(itile_lb[:, q * Q4:(q + 1) * Q4],
                                      I_d[lb][:, q * Q4:(q + 1) * Q4])
            else:
                nc.sync.dma_start(itile_lb[:], I_d[lb])
            itile_lbs[lb] = itile_lb

        def stage_a0(lb, blk):
            """Half-lb u/V/W2 ACT transforms, spread over the lb's 4 steps
            (u first: the PE conv consumes it immediately; V by a4, W2 by
            bh).  Prefetches the next lb's itile at blk 0."""
            pcol = pcol_of(lb)
            t0 = blk * tb
            HT = Tn // 2
            if blk == 0:
                if lb == 0:
                    # startup DMA order: first input quarter, then the g=0
                    # weights the first conv needs, then the rest
                    itile_lb = ipool.tile([PD, Tn], f32, tag="itile")
                    Q4 = Tn // 4
                    nc.sync.dma_start(itile_lb[:, 0:Q4], I_d[0][:, 0:Q4])
                    nc.sync.dma_start(dw[:, 0:10 * PD], dw_d[0][:, 0:10 * PD])
                    nc.sync.dma_start(dw[:, 10 * PD:NW * PD],
                                      dw_d[0][:, 10 * PD:NW * PD])
                    for q in range(1, 4):
                        nc.sync.dma_start(itile_lb[:, q * Q4:(q + 1) * Q4],
                                          I_d[0][:, q * Q4:(q + 1) * Q4])
                    for g_ in range(1, GH):
                        nc.sync.dma_start(
                            dw[:, g_ * NW * PD:(g_ + 1) * NW * PD],
                            dw_d[g_])
                    itile_lbs[0] = itile_lb
                if lb + 1 < nlb:
                    fetch_itile(lb + 1)
                uf = upool.tile([PD, J + Tn], bf16, tag="ufull")
                nc.vector.memset(uf[:, 0:J - 1], 0.0)
                nc.vector.tensor_copy(uf[:, J - 1:J], pcol(CA0))
                ufull_lbs[lb] = uf
                Vt = vpool.tile([PD, Tn], bf16, tag="V")
                W2t = wpool.tile([PD, Tn], bf16, tag="W2")
                vw_lbs[lb] = (Vt, W2t)
            itile = itile_lbs[lb]
            uf = ufull_lbs[lb]
            V, W2 = vw_lbs[lb]
            half = lambda h: slice(h * HT, (h + 1) * HT)

            def aff(out, in_, ci, ca, chunk=False):
                if lb < UVWDVE:
                    if chunk:
                        nc.vector.tensor_scalar(out[:, 0:tb], in_[:, 0:tb],
                                                pcol(ci), pcol(ca),
                                                OP.mult, OP.add)
                        nc.vector.tensor_scalar(out[:, tb:], in_[:, tb:],
                                                pcol(ci), pcol(ca),
                                                OP.mult, OP.add)
                    else:
                        nc.vector.tensor_scalar(out, in_, pcol(ci), pcol(ca),
                                                OP.mult, OP.add)
                else:
                    nc.scalar.activation(out, in_, AF.Identity,
                                         bias=pcol(ca), scale=pcol(ci))

            def w2p(out, in_):
                if lb < UVWDVE:
                    nc.vector.tensor_scalar(out, in_, pcol(SW2), 0.0,
                                            OP.mult, OP.add)
                else:
                    nc.scalar.activation(out, in_, AF.Copy, scale=pcol(SW2))

            if blk == 0:
                aff(uf[:, J:J + HT], itile[:, half(0)], UC, UA, chunk=True)
            elif blk == 1:
                aff(uf[:, J + HT:J + Tn], itile[:, half(1)], UC, UA)
                aff(V[:, half(0)], itile[:, half(0)], SV, AV)
            elif blk == 2:
                aff(V[:, half(1)], itile[:, half(1)], SV, AV)
                w2p(W2[:, half(0)], itile[:, half(0)])
            else:
                w2p(W2[:, half(1)], itile[:, half(1)])
            return V[:, t0:t0 + tb], W2[:, t0:t0 + tb]

        def stage_a1(lb, blk, V, W2):
            """cap0 PE conv."""
            g = lb % GH
            t0 = blk * tb
            uf = ufull_lbs[lb]
            # cap0_t = sum_{j=1..J} C1^{j-1} u_{t-j}  (virtual u_{-1}=Ca'_0)
            cap0 = ps1.tile([PD, tb], f32, tag="cap0")
            for j in range(1, J + 1):
                nc.tensor.matmul(cap0[:], wmat(g, j - 1),
                                 uf[:, J + t0 - j:J + t0 - j + tb],
                                 start=(j == 1), stop=(j == J))
            return cap0, V, W2

        def stage_a2(lb, blk, cap0, V, W2):
            """sig0 + cap0 evacuation (ACT), comb PE matmuls."""
            pcol = pcol_of(lb)
            g = lb % GH
            t0 = blk * tb
            sig = spool.tile([PD, S], bf16, tag="sig")
            sig3 = sig[:].rearrange("p (t k) -> p t k", k=K)
            nc.scalar.activation(sig3[:, :, 0], cap0[:], AF.Sigmoid,
                                 bias=pcol(BIAS), scale=pcol(SC0))
            u_i = blk + nblk * lb
            cap0s = cspool.tile([PD, tb], bf16, tag="cap0s")
            if u_i < EVACDVE:
                nc.vector.tensor_copy(cap0s[:], cap0[:])
            else:
                nc.scalar.activation(cap0s[:], cap0[:], AF.Copy)

            comb = ps2.tile([PD, 2, tb], f32, tag="comb")
            uslice = ufull_lbs[lb][:, J + t0:J + t0 + tb]
            for k in (1, 2):
                nc.tensor.matmul(comb[:, k - 1], wmat(g, 9 + k), cap0s[:],
                                 start=True, stop=False)
                nc.tensor.matmul(comb[:, k - 1], wmat(g, 16 + k), uslice,
                                 start=False, stop=True)
            # comb3 accumulates in place on top of cap0 (slot 12 = G3-1):
            # cap0 + (g3-1)*cap0s + u = g3*cap0 + u (after sig0/evac reads)
            nc.tensor.matmul(cap0[:], wmat(g, 12), cap0s[:],
                             start=False, stop=False, skip_group_check=True)
            nc.tensor.matmul(cap0[:], wmat(g, 16), uslice,
                             start=False, stop=True, skip_group_check=True)
            return sig, sig3, comb, cap0, V, W2

        def stage_a3(lb, blk, sig, sig3, comb, comb3, V, W2):
            """sig1..3 (ACT, all from PSUM; SC1/SC2 folded into PE wts)."""
            pcol = pcol_of(lb)
            nc.scalar.activation(sig3[:, :, 1:3],
                                 comb[:].rearrange("p k t -> p t k"),
                                 AF.Sigmoid, bias=pcol(BIAS), scale=1.0)
            nc.scalar.activation(sig3[:, :, 3], comb3[:], AF.Sigmoid,
                                 bias=pcol(BIAS), scale=pcol(SC3))
            return sig, sig3, V, W2

        def stage_a4(lb, blk, sig, sig3, V, W2):
            """vsig (GPSIMD; DVE for the first units while the b-pipeline
            is still filling and the DVE is idle)."""
            u = blk + nblk * lb
            vsig = bpool.tile([PD, S], bf16, tag="vsig")
            veng = nc.vector if u < VSFILL else nc.gpsimd
            veng.tensor_mul(
                vsig[:].rearrange("p (t k) -> p t k", k=K),
                sig[:].rearrange("p (t k) -> p t k", k=K),
                V[:].unsqueeze(2).broadcast_to((PD, tb, K)))
            return sig, vsig, W2

        def stage_b1(lb, blk, sig, vsig, W2):
            """Pt, Qt, R scan, sr."""
            pcol = pcol_of(lb)
            u = blk + nblk * lb

            Pt = bpool.tile([PD, S], bf16, tag="Pt")
            nc.vector.tensor_scalar(Pt[:], vsig[:], -1.0, pcol(CP),
                                    OP.mult, OP.add)
            Qt = bpool.tile([PD, S], bf16, tag="Qt")
            if u % 20 < QTACT:
                nc.scalar.activation(Qt[:], sig[:], AF.Identity,
                                     bias=pcol(QA), scale=pcol(QM))
            else:
                nc.vector.tensor_scalar(Qt[:], sig[:], pcol(QM), pcol(QA),
                                        OP.mult, OP.add)

            if blk == 0:
                rbig = rpool.tile([PD, nblk * S + 1], bf16, tag="rbig")
                nc.vector.memset(rbig[:, 0:1], 1.0)
                prev_rb[lb] = rbig
            rbig = prev_rb[lb]
            s0 = blk * S
            nc.vector.tensor_tensor_scan(rbig[:, s0 + 1:s0 + S + 1], Pt[:],
                                         Qt[:], rbig[:, s0:s0 + 1],
                                         OP.mult, OP.add)
            return sig, rbig[:, s0:s0 + S], W2

        def stage_b1x(lb, blk, sig, rsh, W2):
            """sr product, one step after the R scan completes."""
            u = blk + nblk * lb
            sr = bpool.tile([PD, S], bf16, tag="sr")
            seng = nc.gpsimd if u % 20 < SRGP else nc.vector
            seng.tensor_mul(sr[:], sig[:], rsh)
            return sr, W2

        def stage_bh(lb, blk, sr, W2):
            """sacc = sum_k e1^{3-k} sr_k on PE; racc = W2*sacc on DVE."""
            g = lb % GH
            u = blk + nblk * lb
            sacc = ps3.tile([PD, tb], f32, tag="sacc")
            srk = sr[:].rearrange("p (t k) -> p t k", k=K)
            for k in range(K):
                nc.tensor.matmul(sacc[:], wmat(g, 13 + k), srk[:, :, k],
                                 start=(k == 0), stop=(k == K - 1))
            racc = cpool.tile([PD, tb], bf16, tag="racc")
            if u % 20 < RACCACT:
                saccs = cpool.tile([PD, tb], bf16, tag="saccs")
                nc.scalar.activation(saccs[:], sacc[:], AF.Copy)
                reng = nc.gpsimd if u % 20 < RACCGP else nc.vector
                reng.tensor_tensor(racc[:], W2[:], saccs[:], OP.mult)
            else:
                nc.vector.tensor_tensor(racc[:], W2[:], sacc[:], OP.mult)
            return racc

        def stage_b2(lb, blk, racc):
            """EPSC scan + out-DMA."""
            pcol = pcol_of(lb)
            etile = cpool.tile([PD, tb], f32, tag="etile")
            einit = 0.0 if blk == 0 else prev_e[lb][:, tb - 1:tb]
            nc.vector.tensor_tensor_scan(
                etile[:], pcol(E14).to_broadcast((PD, tb)), racc[:],
                einit, OP.mult, OP.add)
            prev_e[lb] = etile
            nc.sync.dma_start(O_d[lb][blk][:], etile[:])

        # software pipeline; emission order per step is chosen so that in
        # each engine queue the instructions that free PSUM banks / unblock
        # the PE come first (ACT has no execution-queue lookahead):
        #   a2(i-2): sig0+evac [ACT], comb [PE]
        #   a3(i-3): sig123 [ACT], vsig [GP]
        #   a0(i):   u/V/W2 [ACT]
        #   a1(i-1): cap0 conv [PE]
        #   b1(i-5): Pt/Qt/scan/sr [DVE/GP]
        #   bh(i-7): sacc [PE], racc [DVE]
        #   b2(i-9): EPSC scan + DMA [DVE]
        units = [(lb, blk) for lb in range(nlb) for blk in range(nblk)]
        n = len(units)
        a0o, a1o, a2o, a3o, a4o, b1o, b1xo, bho = ({}, {}, {}, {}, {}, {},
                                                   {}, {})
        for i in range(n + 9):
            if 2 <= i <= n + 1:
                a2o[i - 2] = stage_a2(*units[i - 2], *a1o.pop(i - 2))
            if 3 <= i <= n + 2:
                a3o[i - 3] = stage_a3(*units[i - 3], *a2o.pop(i - 3))
            if i < n:
                a0o[i] = stage_a0(*units[i])
            if 1 <= i <= n:
                a1o[i - 1] = stage_a1(*units[i - 1], *a0o.pop(i - 1))
            if 4 <= i <= n + 3:
                a4o[i - 4] = stage_a4(*units[i - 4], *a3o.pop(i - 4))
            if 5 <= i <= n + 4:
                b1o[i - 5] = stage_b1(*units[i - 5], *a4o.pop(i - 5))
            if 6 <= i <= n + 5:
                b1xo[i - 6] = stage_b1x(*units[i - 6], *b1o.pop(i - 6))
            if 7 <= i <= n + 6:
                bho[i - 7] = stage_bh(*units[i - 7], *b1xo.pop(i - 7))
            if i >= 9:
                stage_b2(*units[i - 9], bho.pop(i - 9))

    import bass_rust
    bass_rust.generate_event_semaphores(nc)
    return nc


def derive_params(log_Ca_mu, log_Ca_sigma, log_tau_Ca, log_alpha, log_tau_EPSC,
                  log_beta, presigmoid_P_rel_max, log_k_recov_min,
                  log_k_recov_delta, ode_steps):
    """Host-side param math (fp64): ([H, NPAR] fp32, [GH, PD, NW*PD] fp32)."""
    d = np.float64
    dt = 1.0 / int(ode_steps)
    mu = np.exp(log_Ca_mu.astype(d))
    sigma = np.exp(log_Ca_sigma.astype(d))
    tau_Ca = np.exp(log_tau_Ca.astype(d))
    alpha = np.exp(log_alpha.astype(d))
    tau_E = np.exp(log_tau_EPSC.astype(d))
    beta = np.exp(log_beta.astype(d))
    Prm = 1.0 / (1.0 + np.exp(-presigmoid_P_rel_max.astype(d)))
    k_min = np.exp(log_k_recov_min.astype(d))
    k_delta = np.exp(log_k_recov_delta.astype(d))

    c1 = 1.0 - dt / tau_Ca
    S1 = np.ones_like(c1)
    S2 = 1.0 + c1
    S3 = 1.0 + c1 + c1 ** 2
    S4 = S3 + c1 ** 3
    e1 = 1.0 - dt / tau_E

    n = log_Ca_mu.shape[0]
    par = np.zeros((n, NPAR), np.float64)
    par[:, C1] = c1 ** 4
    par[:, G1] = c1 * S4 / S1
    par[:, G2] = c1 ** 2 * S4 / S2
    par[:, G3] = c1 ** 3 * S4 / S3
    par[:, SC0] = S4 / sigma
    par[:, SC1] = S1 / sigma
    par[:, SC2] = S2 / sigma
    par[:, SC3] = S3 / sigma
    par[:, BIAS] = -mu / sigma
    par[:, UC] = dt * alpha
    par[:, UA] = dt / tau_Ca * mu
    par[:, SV] = dt * Prm
    par[:, AV] = dt * k_delta
    par[:, CP] = 1.0 - dt * k_min
    par[:, QM] = dt * k_delta
    par[:, QA] = dt * k_min
    par[:, SW2] = -dt * beta * Prm
    par[:, E1] = e1
    par[:, E14] = e1 ** 4
    par[:, CA0] = mu / S4

    C1v = par[:, C1].reshape(GH, PD)
    Gv = np.stack([par[:, G1], par[:, G2], par[:, G3]], -1).reshape(GH, PD, 3)
    e1v = e1.reshape(GH, PD)
    dwm = np.zeros((GH, PD, NW * PD), np.float64)
    idx = np.arange(PD)
    for g in range(GH):
        for j in range(J):
            dwm[g, idx, j * PD + idx] = C1v[g] ** j          # slots 0..9
        SCv = np.stack([par[:, SC1], par[:, SC2]], -1).reshape(GH, PD, 2)
        for k in range(2):
            dwm[g, idx, (10 + k) * PD + idx] = SCv[g, :, k] * Gv[g, :, k]
            dwm[g, idx, (17 + k) * PD + idx] = SCv[g, :, k]
        dwm[g, idx, 12 * PD + idx] = Gv[g, :, 2] - 1.0       # slot 12: G3-1
        for k in range(K):
            dwm[g, idx, (13 + k) * PD + idx] = e1v[g] ** (3 - k)  # 13..16
    return par.astype(np.float32), dwm.astype(np.float32)


_PROG = None
LAST_RESULTS = None  # BassKernelResults of the most recent kernel() call


def _get_program():
    global _PROG
    if _PROG is None:
        _PROG = build_program()
    return _PROG


def _to_bf16(x):
    """fp32 -> bfloat16 numpy array (ml_dtypes)."""
    import ml_dtypes
    return x.astype(ml_dtypes.bfloat16)


def kernel(I_Ca, log_Ca_mu, log_Ca_sigma, log_tau_Ca, log_alpha, log_tau_EPSC,
           log_beta, presigmoid_P_rel_max, log_k_recov_min, log_k_recov_delta,
           ode_steps):
    assert int(ode_steps) == K, f"kernel hardcodes {K} substeps"
    I_Ca = np.asarray(I_Ca, np.float32)
    assert I_Ca.shape == (B, T, H)

    par_h, dwm = derive_params(
        np.asarray(log_Ca_mu), np.asarray(log_Ca_sigma), np.asarray(log_tau_Ca),
        np.asarray(log_alpha), np.asarray(log_tau_EPSC), np.asarray(log_beta),
        np.asarray(presigmoid_P_rel_max), np.asarray(log_k_recov_min),
        np.asarray(log_k_recov_delta), ode_steps)

    # lane-batch lb = b_local*GH + g holds lanes h = g*128 + p
    par_lb = par_h.reshape(GH, PD, NPAR)
    par_core = np.ascontiguousarray(
        np.broadcast_to(par_lb[None], (BPC, GH, PD, NPAR)).reshape(
            NLB, PD, NPAR).transpose(1, 0, 2).reshape(PD, NLB * NPAR))
    dw_core = _to_bf16(np.ascontiguousarray(dwm))

    nc = _get_program()
    in_maps = []
    for c in range(NCORES):
        Ic = I_Ca[c * BPC:(c + 1) * BPC]                    # [BPC, T, H]
        Ic = Ic.reshape(BPC, T, GH, PD).transpose(0, 2, 3, 1)
        in_maps.append({
            "i_ca": np.ascontiguousarray(Ic.reshape(NLB, PD, T)),
            "par": par_core,
            "dw": dw_core,
        })

    res = run_bass_kernel_spmd(nc, in_maps, core_ids=list(range(NCORES)))
    global LAST_RESULTS
    LAST_RESULTS = res
    nblk = T // TB
    out = np.empty((B, T, H), np.float32)
    for c in range(NCORES):
        Oc = np.stack([
            np.concatenate([res.results[c][f"epsc_{lb}_{blk}"]
                            for blk in range(nblk)], axis=1)
            for lb in range(NLB)])                          # [NLB, PD, T]
        Oc = Oc.reshape(BPC, GH, PD, T)
        out[c * BPC:(c + 1) * BPC] = Oc.transpose(0, 3, 1, 2).reshape(BPC, T, H)
    return out
